# revision 8
# baseline (speedup 1.0000x reference)
"""GCN (4-layer GCNConv + BN/ReLU + mean-pool + FC + log_softmax) on 8 Trainium2 NeuronCores.

Sharding: nodes partitioned into 8 cores by contiguous 64-graph windows (graph parallel);
edges partitioned by destination core. Per layer: local matmul -> AllGather of the
dis-scaled feature table -> dma_gather edge aggregation into 4 per-source-chunk
accumulators (ELL-style pass schedule, per-chunk degree-sorted slot orderings) ->
merge -> BN (stats AllReduce) + ReLU. Final: one-hot matmul pooling + FC + log_softmax.
"""
import sys, types, os
import numpy as np


def _install_axon_hooks():
    if "antenv.axon_hooks" in sys.modules:
        return
    try:
        import antenv
    except ImportError:
        return
    mod = types.ModuleType("antenv.axon_hooks")
    state = {"hook": None}
    mod.set_axon_ntff_profile_hook = lambda h: state.__setitem__("hook", h)
    mod.get_axon_ntff_profile_hook = lambda: state["hook"]
    sys.modules["antenv.axon_hooks"] = mod
    antenv.axon_hooks = mod
    try:
        from trn_agent_boot.trn_boot import _ntff_profile_via_ctypes
        state["hook"] = _ntff_profile_via_ctypes("/opt/axon/libaxon_pjrt.so")
    except Exception:
        pass


_install_axon_hooks()

import concourse.bacc as bacc
import concourse.bass as bass
import concourse.mybir as mybir
import concourse.tile as tile
from concourse.ap import AP
from concourse.library_config import mlp
from concourse.bass_utils import run_bass_kernel_spmd

# ---- static problem shapes ----
N = 100000
E = 1600000
G = 512
FIN = 128
H = 64
C = 10
EPS = 1e-5
NCORES = 8
A = 12800            # slots per core (100 blocks of 128)
B = A // 128         # 100 blocks
GW = G // NCORES     # 64 graphs per core
NCHUNK = 4           # source chunks (pairs of cores), 25600 rows each
CHROWS = 2 * A       # rows per source chunk
CALL = 1024          # idxs per dma_gather call (single_packet limit)
NCALLS = 52          # calls per chunk per layer (52*1024 = 53248 >= max padded rows)
MCALLS = 13          # merge gather calls (13*1024 >= 12800)
DUMMY = CHROWS - 1   # in-chunk dummy row index (forced-zero slot 12799 of 2nd core)
PAD_DEG = 1.0e38

LAST_EXEC_NS = None

f32 = mybir.dt.float32
bf16 = mybir.dt.bfloat16
i16 = mybir.dt.int16
Alu = mybir.AluOpType
Act = mybir.ActivationFunctionType


def _wrap_idx(v):
    """int16 idx vector (len mult of 16) -> [128, len/16] wrapped+replicated layout."""
    blk = v.reshape(-1, 16).T.astype(np.int16)
    return np.tile(blk, (8, 1))


def _host_prepare(x, edge_index, batch):
    src = np.asarray(edge_index[0], np.int64)
    dst = np.asarray(edge_index[1], np.int64)
    batch = np.asarray(batch, np.int64)
    gsize = np.bincount(batch, minlength=G)
    gw_nodes = gsize.reshape(NCORES, GW).sum(1)
    assert gw_nodes.max() <= A, f"core node count {gw_nodes.max()} exceeds {A} slots"
    node_off = np.concatenate([[0], np.cumsum(gw_nodes)])
    core_of_node = np.repeat(np.arange(NCORES), gw_nodes)

    dst_core = core_of_node[dst]
    src_chunk = core_of_node[src] // 2

    # per (core, chunk) multiplicity of each node
    cnt = np.zeros((NCHUNK, N), np.int64)
    for q in range(NCHUNK):
        m = src_chunk == q
        cnt[q] += np.bincount(dst[m], minlength=N)
    cnt_tot = cnt.sum(0)

    # orderings: primary = chunk0-sorted; slotq = chunk-q sorted (per core)
    slot_p = np.full(N, -1, np.int64)      # node -> primary slot (0..A)
    node_of_slot = np.full((NCORES, A), -1, np.int64)
    slot_q = np.full((NCHUNK, N), -1, np.int64)
    order_q_all = {}
    for k in range(NCORES):
        nodes_k = np.arange(node_off[k], node_off[k + 1])
        for q in range(NCHUNK):
            order = nodes_k[np.argsort(-cnt[q][nodes_k], kind="stable")]
            slot_q[q][order] = np.arange(len(order))
            order_q_all[(k, q)] = order
        slot_p[order_q_all[(k, 0)]] = np.arange(len(nodes_k))
        node_of_slot[k, :len(nodes_k)] = order_q_all[(k, 0)]

    # common pass lengths L[q][j] (max over cores, 128-aligned)
    npass = np.zeros(NCHUNK, np.int64)
    nact = np.zeros((NCORES, NCHUNK, 64), np.int64)
    for k in range(NCORES):
        for q in range(NCHUNK):
            nodes_k = np.arange(node_off[k], node_off[k + 1])
            cq = cnt[q][nodes_k]
            mx = int(cq.max()) if len(cq) else 0
            npass[q] = max(npass[q], mx)
            for j in range(mx):
                nact[k, q, j] = int((cq > j).sum())
    Lpad = [[int(np.ceil(nact[:, q, j].max() / 128) * 128) for j in range(npass[q])]
            for q in range(NCHUNK)]
    for q in range(NCHUNK):
        assert sum(Lpad[q]) <= NCALLS * CALL, (q, sum(Lpad[q]))

    # gather index arrays [NCHUNK, NCALLS, 128, CALL//16] + add schedule
    idx_all = np.zeros((NCORES, NCHUNK, NCALLS * CALL), np.int16)
    idx_all[:] = DUMMY
    # table row of a source node, within its chunk: (core%2)*A + primary slot
    row_in_chunk = (core_of_node % 2) * A + slot_p
    sched = [[] for _ in range(NCHUNK)]  # per chunk: list of (pos_blk, acc_blk, nblk) common
    for q in range(NCHUNK):
        pos = 0
        for j, L in enumerate(Lpad[q]):
            sched[q].append((pos // 128, 0, L // 128, j))
            pos += L
    for k in range(NCORES):
        ek = dst_core == k
        s_k, d_k = src[ek], dst[ek]
        cq_k = src_chunk[ek]
        for q in range(NCHUNK):
            m = cq_k == q
            s_q, d_q = s_k[m], d_k[m]
            dsl = slot_q[q][d_q]
            o = np.argsort(dsl, kind="stable")
            s_q, dsl = s_q[o], dsl[o]
            # j-th edge of each slot: rank within equal dsl run
            jrank = np.arange(len(dsl)) - np.searchsorted(dsl, dsl)
            pos0 = np.concatenate([[0], np.cumsum([L for L in Lpad[q]])])
            flat = pos0[jrank] + dsl
            idx_all[k, q, flat] = row_in_chunk[s_q].astype(np.int16)

    idx_wrapped = np.zeros((NCORES, NCHUNK, NCALLS, 128, CALL // 16), np.int16)
    for k in range(NCORES):
        for q in range(NCHUNK):
            for c in range(NCALLS):
                idx_wrapped[k, q, c] = _wrap_idx(idx_all[k, q, c * CALL:(c + 1) * CALL])

    # per-call add schedule (common): call c covers staging blocks -> acc block ranges
    addsched = [[[] for _ in range(NCALLS)] for _ in range(NCHUNK)]
    for q in range(NCHUNK):
        pos = 0
        for j, L in enumerate(Lpad[q]):
            for blk in range(L // 128):
                g_abs = pos // 128 + blk
                addsched[q][g_abs // 8].append((g_abs % 8, blk))
            pos += L
    # compress consecutive runs: list of (st_blk0, acc_blk0, n)
    addruns = [[[] for _ in range(NCALLS)] for _ in range(NCHUNK)]
    for q in range(NCHUNK):
        for c in range(NCALLS):
            for st_b, ac_b in addsched[q][c]:
                runs = addruns[q][c]
                if runs and runs[-1][0] + runs[-1][2] == st_b and runs[-1][1] + runs[-1][2] == ac_b:
                    runs[-1][2] += 1
                else:
                    runs.append([st_b, ac_b, 1])

    # merge permutation: z[primary slot s] += acc_q[slot_q of node at s]
    merge_idx = np.zeros((NCORES, NCHUNK - 1, MCALLS, 128, CALL // 16), np.int16)
    for k in range(NCORES):
        nk = int(gw_nodes[k])
        for q in range(1, NCHUNK):
            mi = np.arange(A, dtype=np.int64)
            mi[:nk] = slot_q[q][node_of_slot[k, :nk]]
            mi = np.concatenate([mi, np.zeros(MCALLS * CALL - A, np.int64)])
            for c in range(MCALLS):
                merge_idx[k, q - 1, c] = _wrap_idx(mi[c * CALL:(c + 1) * CALL].astype(np.int16))

    # per-core per-slot data
    xT = np.zeros((NCORES, FIN, A), np.float32)
    deg_loc = np.full((NCORES, 128, B), PAD_DEG, np.float32)
    g_loc = np.full((NCORES, 128, B), -1.0, np.float32)
    for k in range(NCORES):
        nk = int(gw_nodes[k])
        nodes = node_of_slot[k, :nk]
        sl = np.arange(nk)
        xT[k][:, sl] = np.asarray(x, np.float32)[nodes].T
        p, bb = sl % 128, sl // 128
        deg_loc[k][p, bb] = cnt_tot[nodes].astype(np.float32)
        g_loc[k][p, bb] = (batch[nodes] - k * GW).astype(np.float32)

    return dict(gw_nodes=gw_nodes, node_off=node_off, idx_wrapped=idx_wrapped,
                addruns=addruns, merge_idx=merge_idx, xT=xT, deg_loc=deg_loc,
                g_loc=g_loc, Lpad=Lpad)


def _build_program(addruns):
    nc = bacc.Bacc("TRN2", target_bir_lowering=False, debug=False,
                   num_devices=NCORES, num_swdge_queues=4)

    # inputs
    t_xT = nc.dram_tensor("xT", [FIN, A], bf16, kind="ExternalInput")
    t_w1 = nc.dram_tensor("w1", [FIN, H], bf16, kind="ExternalInput")
    t_w = nc.dram_tensor("w234", [H, 3 * H], bf16, kind="ExternalInput")
    t_gb = nc.dram_tensor("gb", [1, 512], f32, kind="ExternalInput")
    t_fcw = nc.dram_tensor("fcw", [H, C], f32, kind="ExternalInput")
    t_fcb = nc.dram_tensor("fcb", [1, C], f32, kind="ExternalInput")
    t_deg = nc.dram_tensor("deg", [128, B], f32, kind="ExternalInput")
    t_gl = nc.dram_tensor("gl", [128, B], f32, kind="ExternalInput")
    t_iota = nc.dram_tensor("iota", [128, H], f32, kind="ExternalInput")
    t_ones = nc.dram_tensor("ones", [1, 128], f32, kind="ExternalInput")
    t_onesc = nc.dram_tensor("onesc", [128, 1], f32, kind="ExternalInput")
    t_onescb = nc.dram_tensor("onescb", [128, 1], bf16, kind="ExternalInput")
    t_ident = nc.dram_tensor("ident", [128, 128], bf16, kind="ExternalInput")
    t_idx = nc.dram_tensor("idx", [NCHUNK, NCALLS, 128, CALL // 16], i16, kind="ExternalInput")
    t_midx = nc.dram_tensor("midx", [NCHUNK - 1, MCALLS, 128, CALL // 16], i16, kind="ExternalInput")
    t_out = nc.dram_tensor("out", [GW, C], f32, kind="ExternalOutput")

    with tile.TileContext(nc) as tc:
        with tc.tile_pool(name="const", bufs=1) as cst, \
             tc.tile_pool(name="accp", bufs=1) as accp, \
             tc.tile_pool(name="work", bufs=1) as wk, \
             tc.tile_pool(name="stage", bufs=8) as stp, \
             tc.tile_pool(name="idxp", bufs=8) as idp, \
             tc.tile_pool(name="small", bufs=2) as smp, \
             tc.tile_pool(name="psA", bufs=2, space="PSUM") as psA, \
             tc.tile_pool(name="psB", bufs=2, space="PSUM") as psB, \
             tc.tile_pool(name="psS", bufs=1, space="PSUM") as psS, \
             tc.tile_pool(name="dram", bufs=1, space="DRAM") as drp:

            nc.gpsimd.load_library(mlp)

            # constants to SBUF
            xT_s = wk.tile([FIN, A], bf16, tag="big2")
            nc.sync.dma_start(xT_s[:], t_xT[:])
            w1_s = cst.tile([FIN, H], bf16)
            nc.sync.dma_start(w1_s[:], t_w1[:])
            w_s = cst.tile([H, 3 * H], bf16)
            nc.sync.dma_start(w_s[:], t_w[:])
            gb_s = cst.tile([1, 512], f32)
            nc.sync.dma_start(gb_s[:], t_gb[:])
            fcw_s = cst.tile([H, C], f32)
            nc.sync.dma_start(fcw_s[:], t_fcw[:])
            fcb_s = cst.tile([1, C], f32)
            nc.sync.dma_start(fcb_s[:], t_fcb[:])
            deg_s = cst.tile([128, B], f32)
            nc.sync.dma_start(deg_s[:], t_deg[:])
            gl_s = cst.tile([128, B], f32)
            nc.sync.dma_start(gl_s[:], t_gl[:])
            iota_s = cst.tile([128, H], f32)
            nc.sync.dma_start(iota_s[:], t_iota[:])
            ones_s = cst.tile([1, 128], f32)
            nc.sync.dma_start(ones_s[:], t_ones[:])
            onesc_s = cst.tile([128, 1], f32)
            nc.sync.dma_start(onesc_s[:], t_onesc[:])
            onescb_s = cst.tile([128, 1], bf16)
            nc.sync.dma_start(onescb_s[:], t_onescb[:])
            ident_s = cst.tile([128, 128], bf16)
            nc.sync.dma_start(ident_s[:], t_ident[:])

            zcol = cst.tile([128, 1], f32)
            nc.vector.memset(zcol[:], 0.0)
            epst = cst.tile([1, 1], f32)
            nc.vector.memset(epst[:], EPS)
            # dis = 1/sqrt(deg+1)
            dis_s = cst.tile([128, B], f32)
            nc.scalar.activation(dis_s[:], deg_s[:], Act.Sqrt, bias=onesc_s[:])
            nc.vector.reciprocal(dis_s[:], dis_s[:])

            def bcast_dis(bsl):  # dis slice [128, nb] -> AP [128, nb, 64]
                s = dis_s[:, bsl]
                return AP(s.tensor, s.offset, [s.ap[0], s.ap[1], [0, H]])

            def rep_free(ap2d, n):  # [P, F] -> [P, n, F] (free repeat)
                return AP(ap2d.tensor, ap2d.offset, [ap2d.ap[0], [0, n], ap2d.ap[1]])

            a_cur = None       # [128, B, H] bf16 post-BN activation (node-major slots)
            aT_cur = None      # [H, A] bf16 transposed

            for l in range(4):
                # ---- local matmul: h_scaled_local = (a @ W) * dis ----
                acc0 = accp.tile([128, B, H], f32, tag="acc0")
                K = FIN if l == 0 else H
                lhsT_full = xT_s if l == 0 else aT_cur
                W_ap = w1_s[:] if l == 0 else w_s[:, (l - 1) * H:l * H]
                for b8 in range(0, B, 8):
                    nblk = min(8, B - b8)
                    pt = psA.tile([128, 8, H], f32, tag="mmps")
                    for bb in range(nblk):
                        nc.tensor.matmul(pt[:, bb], lhsT_full[:, (b8 + bb) * 128:(b8 + bb + 1) * 128],
                                         W_ap, start=True, stop=True)
                    nc.vector.tensor_tensor(out=acc0[:, b8:b8 + nblk], in0=pt[:, 0:nblk],
                                            in1=bcast_dis(slice(b8, b8 + nblk)), op=Alu.mult)
                # shard -> DRAM (row-major by slot: row s=(p + 128*b))
                shard = drp.tile([A, H], f32, tag="shard")
                sh_ap = AP(shard[:].tensor, shard[:].offset,
                           [[H, 128], [128 * H, B], [1, H]])
                nc.sync.dma_start(sh_ap, acc0[:])
                table = drp.tile([NCORES * A, H], f32, tag="table", addr_space="Shared")
                nc.gpsimd.collective_compute(
                    "AllGather", Alu.bypass,
                    replica_groups=[list(range(NCORES))],
                    ins=[shard[:]], outs=[table[:]])

                # ---- edge gathers into 4 accumulators ----
                accq = [acc0]
                for q in range(1, NCHUNK):
                    aq = accp.tile([128, B, H], f32, tag=f"acc{q}")
                    nc.vector.memset(aq[:], 0.0)
                    accq.append(aq)
                for q in range(NCHUNK):
                    src_ap = table[q * CHROWS:(q + 1) * CHROWS, :]
                    for c in range(NCALLS):
                        if not addruns[q][c]:
                            continue
                        it = idp.tile([128, CALL // 16], i16, tag="idx")
                        nc.sync.dma_start(it[:], t_idx[q, c])
                        st = stp.tile([128, 8, H], f32, tag="stage")
                        nc.gpsimd.dma_gather(st[:], src_ap, it[:], CALL, CALL, H,
                                             single_packet=True, queue_num=q)
                        for st_b, ac_b, nb in addruns[q][c]:
                            nc.vector.tensor_add(accq[q][:, ac_b:ac_b + nb],
                                                 accq[q][:, ac_b:ac_b + nb],
                                                 st[:, st_b:st_b + nb])
                # ---- merge acc1..3 into acc0 (permuted via local gather) ----
                for q in range(1, NCHUNK):
                    scr = drp.tile([A, H], f32, tag=f"scr{q}")
                    sc_ap = AP(scr[:].tensor, scr[:].offset,
                               [[H, 128], [128 * H, B], [1, H]])
                    nc.sync.dma_start(sc_ap, accq[q][:])
                    for c in range(MCALLS):
                        nblk = min(8, B - c * 8)
                        it = idp.tile([128, CALL // 16], i16, tag="idx")
                        nc.sync.dma_start(it[:], t_midx[q - 1, c])
                        st = stp.tile([128, 8, H], f32, tag="stage")
                        nc.gpsimd.dma_gather(st[:], scr[:], it[:], CALL, CALL, H,
                                             single_packet=True, queue_num=q)
                        nc.vector.tensor_add(acc0[:, c * 8:c * 8 + nblk],
                                             acc0[:, c * 8:c * 8 + nblk], st[:, 0:nblk])

                # ---- zt = acc0 * dis ; stats; BN+ReLU ----
                nc.vector.tensor_tensor(out=acc0[:], in0=acc0[:], in1=bcast_dis(slice(0, B)), op=Alu.mult)
                zt = acc0
                s1t = smp.tile([128, H], f32, tag="s1")
                nc.vector.tensor_reduce(s1t[:], zt[:].rearrange("p b f -> p f b"),
                                        axis=mybir.AxisListType.X, op=Alu.add)
                sq = wk.tile([128, B, H], f32, tag="big2")
                nc.vector.tensor_mul(sq[:], zt[:], zt[:])
                s2t = smp.tile([128, H], f32, tag="s2")
                nc.vector.tensor_reduce(s2t[:], sq[:].rearrange("p b f -> p f b"),
                                        axis=mybir.AxisListType.X, op=Alu.add)
                spt = psS.tile([1, 128], f32, tag="stps")
                nc.tensor.matmul(spt[:, 0:H], onesc_s[:], s1t[:], start=True, stop=True)
                nc.tensor.matmul(spt[:, H:2 * H], onesc_s[:], s2t[:], start=True, stop=True)
                srow = smp.tile([1, 128], f32, tag="srow")
                nc.vector.tensor_copy(srow[:], spt[:])
                arb_in = drp.tile([1, 128], f32, tag="arbin")
                arb_out = drp.tile([1, 128], f32, tag="arbout", addr_space="Shared")
                nc.sync.dma_start(arb_in[:], srow[:])
                nc.gpsimd.collective_compute(
                    "AllReduce", Alu.add,
                    replica_groups=[list(range(NCORES))],
                    ins=[arb_in[:]], outs=[arb_out[:]])
                sg = smp.tile([1, 128], f32, tag="sg")
                nc.sync.dma_start(sg[:], arb_out[:])
                # m=sg[0:64]/N; ex2=sg[64:128]/N; var=ex2-m*m; rs=1/sqrt(var+eps)
                mrow = smp.tile([1, H], f32, tag="mrow")
                nc.vector.tensor_scalar_mul(mrow[:], sg[:, 0:H], 1.0 / N)
                vrow = smp.tile([1, H], f32, tag="vrow")
                nc.vector.tensor_scalar_mul(vrow[:], sg[:, H:2 * H], 1.0 / N)
                mm = smp.tile([1, H], f32, tag="mm")
                nc.vector.tensor_mul(mm[:], mrow[:], mrow[:])
                nc.vector.tensor_sub(vrow[:], vrow[:], mm[:])
                nc.scalar.activation(vrow[:], vrow[:], Act.Sqrt, bias=epst[:])
                nc.vector.reciprocal(vrow[:], vrow[:])          # rs
                arow = smp.tile([1, H], f32, tag="arow")
                nc.vector.tensor_mul(arow[:], vrow[:], gb_s[:, l * H:(l + 1) * H])   # alpha
                crow = smp.tile([1, H], f32, tag="crow")
                nc.vector.tensor_mul(crow[:], mrow[:], arow[:])
                nc.vector.tensor_sub(crow[:], gb_s[:, 256 + l * H:256 + (l + 1) * H], crow[:])  # c2
                bcp = psS.tile([128, 2 * H], f32, tag="bcps")
                nc.tensor.matmul(bcp[:, 0:H], ones_s[:], arow[:], start=True, stop=True)
                nc.tensor.matmul(bcp[:, H:2 * H], ones_s[:], crow[:], start=True, stop=True)
                abc = smp.tile([128, 2 * H], f32, tag="abc")
                nc.vector.tensor_copy(abc[:], bcp[:])
                # a = relu(zt*alpha + c2)  (bf16)
                nc.vector.tensor_tensor(out=acc0[:], in0=zt[:], in1=rep_free(abc[:, 0:H], B), op=Alu.mult)
                nc.vector.tensor_tensor(out=acc0[:], in0=acc0[:], in1=rep_free(abc[:, H:2 * H], B), op=Alu.add)
                a_cur = wk.tile([128, B, H], bf16, tag="a")
                nc.scalar.activation(a_cur[:], acc0[:], Act.Relu, bias=zcol[:])
                nc.vector.memset(a_cur[96:128, B - 1:B], 0.0)   # forced-zero pad slots incl 12799

                if l < 3:
                    # transpose a -> aT [H, A] bf16
                    aT_cur = wk.tile([H, A], bf16, tag="big2")
                    for b4 in range(0, B, 4):
                        nblk = min(4, B - b4)
                        tp = psB.tile([H, 4, 128], bf16, tag="trps")
                        for bb in range(nblk):
                            nc.tensor.transpose(tp[:, bb], a_cur[:, b4 + bb], ident_s[:])
                        nc.vector.tensor_copy(aT_cur[:, (b4) * 128:(b4 + nblk) * 128],
                                              tp[:, 0:nblk].rearrange("p b f -> p (b f)"))

            # ---- pooling: per-graph mean over this core's 64 graphs ----
            pc = psS.tile([GW, H + 1], f32, tag="poolcnt")
            poolp = pc[:, 0:H]
            cntp = pc[:, H:H + 1]
            for b in range(B):
                oh = smp.tile([128, H], bf16, tag="oh", bufs=4)
                nc.vector.tensor_scalar(oh[:], iota_s[:], gl_s[:, b:b + 1], None, Alu.is_equal)
                nc.tensor.matmul(poolp, oh[:], a_cur[:, b], start=(b == 0), stop=(b == B - 1))
                nc.tensor.matmul(cntp, oh[:], onescb_s[:], start=(b == 0), stop=(b == B - 1))
            sums = smp.tile([GW, H], f32, tag="sums")
            nc.vector.tensor_copy(sums[:], poolp)
            cnts = smp.tile([GW, 1], f32, tag="cnts")
            nc.vector.tensor_copy(cnts[:], cntp)
            nc.vector.tensor_scalar_max(cnts[:], cnts[:], 1.0)
            nc.vector.reciprocal(cnts[:], cnts[:])
            nc.vector.tensor_scalar(sums[:], sums[:], cnts[:], None, Alu.mult)
            # pooledT
            ptp = psS.tile([H, GW], f32, tag="stps")
            identf = smp.tile([128, 128], f32, tag="identf")
            nc.vector.tensor_copy(identf[:], ident_s[:])
            nc.tensor.transpose(ptp[:], sums[:], identf[0:GW, 0:GW])
            pooledT = smp.tile([H, GW], f32, tag="pooledT")
            nc.vector.tensor_copy(pooledT[:], ptp[:])
            # logits = fcb + pooled @ fcW
            lgp = psS.tile([GW, C], f32, tag="bcps")
            nc.tensor.matmul(lgp[:], ones_s[:, 0:GW], fcb_s[:], start=True, stop=False)
            nc.tensor.matmul(lgp[:], pooledT[:], fcw_s[:], start=False, stop=True)
            logits = smp.tile([GW, C], f32, tag="logits")
            nc.vector.tensor_copy(logits[:], lgp[:])
            mx = smp.tile([GW, 1], f32, tag="mx")
            nc.vector.tensor_reduce(mx[:], logits[:], axis=mybir.AxisListType.X, op=Alu.max)
            nmx = smp.tile([GW, 1], f32, tag="nmx")
            nc.vector.tensor_scalar_mul(nmx[:], mx[:], -1.0)
            et = smp.tile([GW, C], f32, tag="et")
            sume = smp.tile([GW, 1], f32, tag="sume")
            nc.scalar.activation(et[:], logits[:], Act.Exp, bias=nmx[:], accum_out=sume[:])
            lse = smp.tile([GW, 1], f32, tag="lse")
            nc.scalar.activation(lse[:], sume[:], Act.Ln, bias=zcol[0:GW, :])
            res = smp.tile([GW, C], f32, tag="res")
            nc.vector.tensor_scalar(res[:], logits[:], mx[:], lse[:], Alu.subtract, Alu.subtract)
            nc.sync.dma_start(t_out[:], res[:])

    nc.compile()
    return nc


def kernel(x, edge_index, batch, W1, b1, g1, bt1, W2, b2, g2, bt2,
           W3, b3, g3, bt3, W4, b4, g4, bt4, fcW, fcb, **_unused):
    global LAST_EXEC_NS
    hp = _host_prepare(x, edge_index, batch)
    nc = _build_program(hp["addruns"])

    gb = np.zeros((1, 512), np.float32)
    for i, (g, bt) in enumerate(((g1, bt1), (g2, bt2), (g3, bt3), (g4, bt4))):
        gb[0, i * H:(i + 1) * H] = np.asarray(g, np.float32)
        gb[0, 256 + i * H:256 + (i + 1) * H] = np.asarray(bt, np.float32)
    w234 = np.concatenate([np.asarray(w, np.float32) for w in (W2, W3, W4)], axis=1)
    iota = np.tile(np.arange(H, dtype=np.float32)[None, :], (128, 1))
    common = {
        "w1": np.asarray(W1, np.float32).astype(np.dtype("bfloat16") if False else np.float32),
        "gb": gb, "fcw": np.asarray(fcW, np.float32), "fcb": np.asarray(fcb, np.float32).reshape(1, C),
        "iota": iota,
        "ones": np.ones((1, 128), np.float32),
        "onesc": np.ones((128, 1), np.float32),
        "ident": np.eye(128, dtype=np.float32),
    }
    import ml_dtypes
    bfl = ml_dtypes.bfloat16
    in_maps = []
    for k in range(NCORES):
        m = {
            "xT": hp["xT"][k].astype(bfl),
            "w1": np.asarray(W1, np.float32).astype(bfl),
            "w234": w234.astype(bfl),
            "gb": gb, "fcw": common["fcw"], "fcb": common["fcb"],
            "deg": hp["deg_loc"][k], "gl": hp["g_loc"][k],
            "iota": iota, "ones": common["ones"], "onesc": common["onesc"],
            "onescb": np.ones((128, 1), bfl),
            "ident": np.eye(128, dtype=np.float32).astype(bfl),
            "idx": hp["idx_wrapped"][k], "midx": hp["merge_idx"][k],
        }
        in_maps.append(m)

    res = run_bass_kernel_spmd(nc, in_maps, core_ids=list(range(NCORES)),
                               trace=os.environ.get("GCN_TRACE", "0") == "1")
    LAST_EXEC_NS = res.exec_time_ns
    out = np.concatenate([res.results[k]["out"] for k in range(NCORES)], axis=0)
    return out.astype(np.float32)


# revision 9
# speedup vs baseline: 2.2884x; 2.2884x over previous
"""GCN (4-layer GCNConv + BN/ReLU + mean-pool + FC + log_softmax) on 8 Trainium2 NeuronCores.

Sharding: nodes partitioned into 8 cores by contiguous 64-graph windows (graph parallel);
edges partitioned by destination core. Per layer: local matmul -> AllGather of the
dis-scaled feature table -> dma_gather edge aggregation into 4 per-source-chunk
accumulators (ELL-style pass schedule, per-chunk degree-sorted slot orderings) ->
merge -> BN (stats AllReduce) + ReLU. Final: one-hot matmul pooling + FC + log_softmax.
"""
import sys, types, os
import numpy as np


def _install_axon_hooks():
    if "antenv.axon_hooks" in sys.modules:
        return
    try:
        import antenv
    except ImportError:
        return
    mod = types.ModuleType("antenv.axon_hooks")
    state = {"hook": None}
    mod.set_axon_ntff_profile_hook = lambda h: state.__setitem__("hook", h)
    mod.get_axon_ntff_profile_hook = lambda: state["hook"]
    sys.modules["antenv.axon_hooks"] = mod
    antenv.axon_hooks = mod
    try:
        from trn_agent_boot.trn_boot import _ntff_profile_via_ctypes
        state["hook"] = _ntff_profile_via_ctypes("/opt/axon/libaxon_pjrt.so")
    except Exception:
        pass


_install_axon_hooks()

import concourse.bacc as bacc
import concourse.bass as bass
import concourse.mybir as mybir
import concourse.tile as tile
from concourse.ap import AP
from concourse.library_config import mlp
from concourse.bass_utils import run_bass_kernel_spmd

# ---- static problem shapes ----
N = 100000
E = 1600000
G = 512
FIN = 128
H = 64
C = 10
EPS = 1e-5
NCORES = 8
A = 12800            # slots per core (100 blocks of 128)
B = A // 128         # 100 blocks
GW = G // NCORES     # 64 graphs per core
NCHUNK = 4           # source chunks (pairs of cores), 25600 rows each
CHROWS = 2 * A       # rows per source chunk
CALL = 1024          # idxs per dma_gather call (single_packet limit)
NCALLS = 52          # calls per chunk per layer (52*1024 = 53248 >= max padded rows)
MCALLS = 13          # merge gather calls (13*1024 >= 12800)
DUMMY = CHROWS - 1   # in-chunk dummy row index (forced-zero slot 12799 of 2nd core)
PAD_DEG = 1.0e38

LAST_EXEC_NS = None

f32 = mybir.dt.float32
bf16 = mybir.dt.bfloat16
i16 = mybir.dt.int16
Alu = mybir.AluOpType
Act = mybir.ActivationFunctionType


def _wrap_idx(v):
    """int16 idx vector (len mult of 16) -> [128, len/16] wrapped+replicated layout."""
    blk = v.reshape(-1, 16).T.astype(np.int16)
    return np.tile(blk, (8, 1))


def _host_prepare(x, edge_index, batch):
    src = np.asarray(edge_index[0], np.int64)
    dst = np.asarray(edge_index[1], np.int64)
    batch = np.asarray(batch, np.int64)
    gsize = np.bincount(batch, minlength=G)
    gw_nodes = gsize.reshape(NCORES, GW).sum(1)
    assert gw_nodes.max() <= A, f"core node count {gw_nodes.max()} exceeds {A} slots"
    node_off = np.concatenate([[0], np.cumsum(gw_nodes)])
    core_of_node = np.repeat(np.arange(NCORES), gw_nodes)

    dst_core = core_of_node[dst]
    src_chunk = core_of_node[src] // 2

    # per (core, chunk) multiplicity of each node
    cnt = np.zeros((NCHUNK, N), np.int64)
    for q in range(NCHUNK):
        m = src_chunk == q
        cnt[q] += np.bincount(dst[m], minlength=N)
    cnt_tot = cnt.sum(0)

    # orderings: primary = chunk0-sorted; slotq = chunk-q sorted (per core)
    slot_p = np.full(N, -1, np.int64)      # node -> primary slot (0..A)
    node_of_slot = np.full((NCORES, A), -1, np.int64)
    slot_q = np.full((NCHUNK, N), -1, np.int64)
    order_q_all = {}
    for k in range(NCORES):
        nodes_k = np.arange(node_off[k], node_off[k + 1])
        for q in range(NCHUNK):
            order = nodes_k[np.argsort(-cnt[q][nodes_k], kind="stable")]
            slot_q[q][order] = np.arange(len(order))
            order_q_all[(k, q)] = order
        slot_p[order_q_all[(k, 0)]] = np.arange(len(nodes_k))
        node_of_slot[k, :len(nodes_k)] = order_q_all[(k, 0)]

    # common pass lengths L[q][j] (max over cores, 128-aligned)
    npass = np.zeros(NCHUNK, np.int64)
    nact = np.zeros((NCORES, NCHUNK, 64), np.int64)
    for k in range(NCORES):
        for q in range(NCHUNK):
            nodes_k = np.arange(node_off[k], node_off[k + 1])
            cq = cnt[q][nodes_k]
            mx = int(cq.max()) if len(cq) else 0
            npass[q] = max(npass[q], mx)
            for j in range(mx):
                nact[k, q, j] = int((cq > j).sum())
    Lpad = [[int(np.ceil(nact[:, q, j].max() / 128) * 128) for j in range(npass[q])]
            for q in range(NCHUNK)]
    for q in range(NCHUNK):
        assert sum(Lpad[q]) <= NCALLS * CALL, (q, sum(Lpad[q]))

    # gather index arrays [NCHUNK, NCALLS, 128, CALL//16] + add schedule
    idx_all = np.zeros((NCORES, NCHUNK, NCALLS * CALL), np.int16)
    idx_all[:] = DUMMY
    # table row of a source node, within its chunk: (core%2)*A + primary slot
    row_in_chunk = (core_of_node % 2) * A + slot_p
    sched = [[] for _ in range(NCHUNK)]  # per chunk: list of (pos_blk, acc_blk, nblk) common
    for q in range(NCHUNK):
        pos = 0
        for j, L in enumerate(Lpad[q]):
            sched[q].append((pos // 128, 0, L // 128, j))
            pos += L
    for k in range(NCORES):
        ek = dst_core == k
        s_k, d_k = src[ek], dst[ek]
        cq_k = src_chunk[ek]
        for q in range(NCHUNK):
            m = cq_k == q
            s_q, d_q = s_k[m], d_k[m]
            dsl = slot_q[q][d_q]
            o = np.argsort(dsl, kind="stable")
            s_q, dsl = s_q[o], dsl[o]
            # j-th edge of each slot: rank within equal dsl run
            jrank = np.arange(len(dsl)) - np.searchsorted(dsl, dsl)
            pos0 = np.concatenate([[0], np.cumsum([L for L in Lpad[q]])])
            flat = pos0[jrank] + dsl
            idx_all[k, q, flat] = row_in_chunk[s_q].astype(np.int16)

    idx_wrapped = np.zeros((NCORES, NCHUNK, NCALLS, 128, CALL // 16), np.int16)
    for k in range(NCORES):
        for q in range(NCHUNK):
            for c in range(NCALLS):
                idx_wrapped[k, q, c] = _wrap_idx(idx_all[k, q, c * CALL:(c + 1) * CALL])

    # per-call add schedule (common): call c covers staging blocks -> acc block ranges
    addsched = [[[] for _ in range(NCALLS)] for _ in range(NCHUNK)]
    for q in range(NCHUNK):
        pos = 0
        for j, L in enumerate(Lpad[q]):
            for blk in range(L // 128):
                g_abs = pos // 128 + blk
                addsched[q][g_abs // 8].append((g_abs % 8, blk))
            pos += L
    # compress consecutive runs: list of (st_blk0, acc_blk0, n)
    addruns = [[[] for _ in range(NCALLS)] for _ in range(NCHUNK)]
    for q in range(NCHUNK):
        for c in range(NCALLS):
            for st_b, ac_b in addsched[q][c]:
                runs = addruns[q][c]
                if runs and runs[-1][0] + runs[-1][2] == st_b and runs[-1][1] + runs[-1][2] == ac_b:
                    runs[-1][2] += 1
                else:
                    runs.append([st_b, ac_b, 1])

    # merge permutation: z[primary slot s] += acc_q[slot_q of node at s]
    merge_idx = np.zeros((NCORES, NCHUNK - 1, MCALLS, 128, CALL // 16), np.int16)
    for k in range(NCORES):
        nk = int(gw_nodes[k])
        for q in range(1, NCHUNK):
            mi = np.arange(A, dtype=np.int64)
            mi[:nk] = slot_q[q][node_of_slot[k, :nk]]
            mi = np.concatenate([mi, np.zeros(MCALLS * CALL - A, np.int64)])
            for c in range(MCALLS):
                merge_idx[k, q - 1, c] = _wrap_idx(mi[c * CALL:(c + 1) * CALL].astype(np.int16))

    # per-core per-slot data
    xT = np.zeros((NCORES, FIN, A), np.float32)
    deg_loc = np.full((NCORES, 128, B), PAD_DEG, np.float32)
    g_loc = np.full((NCORES, 128, B), -1.0, np.float32)
    for k in range(NCORES):
        nk = int(gw_nodes[k])
        nodes = node_of_slot[k, :nk]
        sl = np.arange(nk)
        xT[k][:, sl] = np.asarray(x, np.float32)[nodes].T
        p, bb = sl % 128, sl // 128
        deg_loc[k][p, bb] = cnt_tot[nodes].astype(np.float32)
        g_loc[k][p, bb] = (batch[nodes] - k * GW).astype(np.float32)

    return dict(gw_nodes=gw_nodes, node_off=node_off, idx_wrapped=idx_wrapped,
                addruns=addruns, merge_idx=merge_idx, xT=xT, deg_loc=deg_loc,
                g_loc=g_loc, Lpad=Lpad)


def _build_program(addruns):
    nc = bacc.Bacc("TRN2", target_bir_lowering=False, debug=False,
                   num_devices=NCORES, num_swdge_queues=4)

    # inputs
    t_xT = nc.dram_tensor("xT", [FIN, A], bf16, kind="ExternalInput")
    t_w1 = nc.dram_tensor("w1", [FIN, H], bf16, kind="ExternalInput")
    t_w = nc.dram_tensor("w234", [H, 3 * H], bf16, kind="ExternalInput")
    t_gb = nc.dram_tensor("gb", [1, 512], f32, kind="ExternalInput")
    t_fcw = nc.dram_tensor("fcw", [H, C], f32, kind="ExternalInput")
    t_fcb = nc.dram_tensor("fcb", [1, C], f32, kind="ExternalInput")
    t_deg = nc.dram_tensor("deg", [128, B], f32, kind="ExternalInput")
    t_gl = nc.dram_tensor("gl", [128, B], f32, kind="ExternalInput")
    t_iota = nc.dram_tensor("iota", [128, H], f32, kind="ExternalInput")
    t_ones = nc.dram_tensor("ones", [1, 128], f32, kind="ExternalInput")
    t_onesc = nc.dram_tensor("onesc", [128, 1], f32, kind="ExternalInput")
    t_onescb = nc.dram_tensor("onescb", [128, 1], bf16, kind="ExternalInput")
    t_ident = nc.dram_tensor("ident", [128, 128], bf16, kind="ExternalInput")
    t_idx = nc.dram_tensor("idx", [NCHUNK, NCALLS, 128, CALL // 16], i16, kind="ExternalInput")
    t_midx = nc.dram_tensor("midx", [NCHUNK - 1, MCALLS, 128, CALL // 16], i16, kind="ExternalInput")
    t_out = nc.dram_tensor("out", [GW, C], f32, kind="ExternalOutput")

    with tile.TileContext(nc) as tc:
        with tc.tile_pool(name="const", bufs=1) as cst, \
             tc.tile_pool(name="accp", bufs=1) as accp, \
             tc.tile_pool(name="work", bufs=1) as wk, \
             tc.tile_pool(name="stage", bufs=8) as stp, \
             tc.tile_pool(name="idxp", bufs=8) as idp, \
             tc.tile_pool(name="small", bufs=2) as smp, \
             tc.tile_pool(name="psA", bufs=2, space="PSUM") as psA, \
             tc.tile_pool(name="psB", bufs=2, space="PSUM") as psB, \
             tc.tile_pool(name="psS", bufs=1, space="PSUM") as psS, \
             tc.tile_pool(name="dram", bufs=1, space="DRAM") as drp:

            nc.gpsimd.load_library(mlp)

            # constants to SBUF
            xT_s = wk.tile([FIN, A], bf16, tag="big2")
            nc.sync.dma_start(xT_s[:], t_xT[:])
            w1_s = cst.tile([FIN, H], bf16)
            nc.sync.dma_start(w1_s[:], t_w1[:])
            w_s = cst.tile([H, 3 * H], bf16)
            nc.sync.dma_start(w_s[:], t_w[:])
            gb_s = cst.tile([1, 512], f32)
            nc.sync.dma_start(gb_s[:], t_gb[:])
            fcw_s = cst.tile([H, C], f32)
            nc.sync.dma_start(fcw_s[:], t_fcw[:])
            fcb_s = cst.tile([1, C], f32)
            nc.sync.dma_start(fcb_s[:], t_fcb[:])
            deg_s = cst.tile([128, B], f32)
            nc.sync.dma_start(deg_s[:], t_deg[:])
            gl_s = cst.tile([128, B], f32)
            nc.sync.dma_start(gl_s[:], t_gl[:])
            iota_s = cst.tile([128, H], f32)
            nc.sync.dma_start(iota_s[:], t_iota[:])
            ones_s = cst.tile([1, 128], f32)
            nc.sync.dma_start(ones_s[:], t_ones[:])
            onesc_s = cst.tile([128, 1], f32)
            nc.sync.dma_start(onesc_s[:], t_onesc[:])
            onescb_s = cst.tile([128, 1], bf16)
            nc.sync.dma_start(onescb_s[:], t_onescb[:])
            ident_s = cst.tile([128, 128], bf16)
            nc.sync.dma_start(ident_s[:], t_ident[:])

            zcol = cst.tile([128, 1], f32)
            nc.vector.memset(zcol[:], 0.0)
            epst = cst.tile([1, 1], f32)
            nc.vector.memset(epst[:], EPS)
            # dis = 1/sqrt(deg+1)
            dis_s = cst.tile([128, B], f32)
            nc.scalar.activation(dis_s[:], deg_s[:], Act.Sqrt, bias=onesc_s[:])
            nc.vector.reciprocal(dis_s[:], dis_s[:])

            def bcast_dis(bsl):  # dis slice [128, nb] -> AP [128, nb, 64]
                s = dis_s[:, bsl]
                return AP(s.tensor, s.offset, [s.ap[0], s.ap[1], [0, H]])

            def rep_free(ap2d, n):  # [P, F] -> [P, n, F] (free repeat)
                return AP(ap2d.tensor, ap2d.offset, [ap2d.ap[0], [0, n], ap2d.ap[1]])

            a_cur = None       # [128, B, H] bf16 post-BN activation (node-major slots)
            aT_cur = None      # [H, A] bf16 transposed

            for l in range(4):
                # ---- local matmul: h_scaled_local = (a @ W) * dis ----
                acc0 = accp.tile([128, B, H], f32, tag="acc0")
                K = FIN if l == 0 else H
                lhsT_full = xT_s if l == 0 else aT_cur
                W_ap = w1_s[:] if l == 0 else w_s[:, (l - 1) * H:l * H]
                for b8 in range(0, B, 8):
                    nblk = min(8, B - b8)
                    pt = psA.tile([128, 8, H], f32, tag="mmps")
                    for bb in range(nblk):
                        nc.tensor.matmul(pt[:, bb], lhsT_full[:, (b8 + bb) * 128:(b8 + bb + 1) * 128],
                                         W_ap, start=True, stop=True)
                    nc.vector.tensor_tensor(out=acc0[:, b8:b8 + nblk], in0=pt[:, 0:nblk],
                                            in1=bcast_dis(slice(b8, b8 + nblk)), op=Alu.mult)
                # shard -> DRAM (row-major by slot: row s=(p + 128*b))
                shard = drp.tile([A, H], f32, tag="shard")
                sh_ap = AP(shard[:].tensor, shard[:].offset,
                           [[H, 128], [128 * H, B], [1, H]])
                nc.sync.dma_start(sh_ap, acc0[:])
                table = drp.tile([NCORES * A, H], f32, tag="table", addr_space="Shared")
                nc.gpsimd.collective_compute(
                    "AllGather", Alu.bypass,
                    replica_groups=[list(range(NCORES))],
                    ins=[shard[:]], outs=[table[:]])

                # ---- edge gathers into 4 accumulators ----
                accq = [acc0]
                for q in range(1, NCHUNK):
                    aq = accp.tile([128, B, H], f32, tag=f"acc{q}")
                    nc.vector.memset(aq[:], 0.0)
                    accq.append(aq)
                for c in range(NCALLS):
                    for q in range(NCHUNK):
                        if not addruns[q][c]:
                            continue
                        src_ap = table[q * CHROWS:(q + 1) * CHROWS, :]
                        it = idp.tile([128, CALL // 16], i16, tag="idx")
                        nc.sync.dma_start(it[:], t_idx[q, c])
                        st = stp.tile([128, 8, H], f32, tag="stage")
                        nc.gpsimd.dma_gather(st[:], src_ap, it[:], CALL, CALL, H,
                                             single_packet=True, queue_num=q)
                        for st_b, ac_b, nb in addruns[q][c]:
                            nc.vector.tensor_add(accq[q][:, ac_b:ac_b + nb],
                                                 accq[q][:, ac_b:ac_b + nb],
                                                 st[:, st_b:st_b + nb])
                # ---- merge acc1..3 into acc0 (permuted via local gather) ----
                scrs = {}
                for q in range(1, NCHUNK):
                    scr = drp.tile([A, H], f32, tag=f"scr{q}")
                    sc_ap = AP(scr[:].tensor, scr[:].offset,
                               [[H, 128], [128 * H, B], [1, H]])
                    nc.sync.dma_start(sc_ap, accq[q][:])
                    scrs[q] = scr
                for c in range(MCALLS):
                    for q in range(1, NCHUNK):
                        nblk = min(8, B - c * 8)
                        it = idp.tile([128, CALL // 16], i16, tag="idx")
                        nc.sync.dma_start(it[:], t_midx[q - 1, c])
                        st = stp.tile([128, 8, H], f32, tag="stage")
                        nc.gpsimd.dma_gather(st[:], scrs[q][:], it[:], CALL, CALL, H,
                                             single_packet=True, queue_num=(q + c) % 4)
                        nc.vector.tensor_add(acc0[:, c * 8:c * 8 + nblk],
                                             acc0[:, c * 8:c * 8 + nblk], st[:, 0:nblk])

                # ---- zt = acc0 * dis ; stats; BN+ReLU ----
                nc.vector.tensor_tensor(out=acc0[:], in0=acc0[:], in1=bcast_dis(slice(0, B)), op=Alu.mult)
                zt = acc0
                s1t = smp.tile([128, H], f32, tag="s1")
                nc.vector.tensor_reduce(s1t[:], zt[:].rearrange("p b f -> p f b"),
                                        axis=mybir.AxisListType.X, op=Alu.add)
                sq = wk.tile([128, B, H], f32, tag="big2")
                nc.vector.tensor_mul(sq[:], zt[:], zt[:])
                s2t = smp.tile([128, H], f32, tag="s2")
                nc.vector.tensor_reduce(s2t[:], sq[:].rearrange("p b f -> p f b"),
                                        axis=mybir.AxisListType.X, op=Alu.add)
                spt = psS.tile([1, 128], f32, tag="stps")
                nc.tensor.matmul(spt[:, 0:H], onesc_s[:], s1t[:], start=True, stop=True)
                nc.tensor.matmul(spt[:, H:2 * H], onesc_s[:], s2t[:], start=True, stop=True)
                srow = smp.tile([1, 128], f32, tag="srow")
                nc.vector.tensor_copy(srow[:], spt[:])
                arb_in = drp.tile([1, 128], f32, tag="arbin")
                arb_out = drp.tile([1, 128], f32, tag="arbout", addr_space="Shared")
                nc.sync.dma_start(arb_in[:], srow[:])
                nc.gpsimd.collective_compute(
                    "AllReduce", Alu.add,
                    replica_groups=[list(range(NCORES))],
                    ins=[arb_in[:]], outs=[arb_out[:]])
                sg = smp.tile([1, 128], f32, tag="sg")
                nc.sync.dma_start(sg[:], arb_out[:])
                # m=sg[0:64]/N; ex2=sg[64:128]/N; var=ex2-m*m; rs=1/sqrt(var+eps)
                mrow = smp.tile([1, H], f32, tag="mrow")
                nc.vector.tensor_scalar_mul(mrow[:], sg[:, 0:H], 1.0 / N)
                vrow = smp.tile([1, H], f32, tag="vrow")
                nc.vector.tensor_scalar_mul(vrow[:], sg[:, H:2 * H], 1.0 / N)
                mm = smp.tile([1, H], f32, tag="mm")
                nc.vector.tensor_mul(mm[:], mrow[:], mrow[:])
                nc.vector.tensor_sub(vrow[:], vrow[:], mm[:])
                nc.scalar.activation(vrow[:], vrow[:], Act.Sqrt, bias=epst[:])
                nc.vector.reciprocal(vrow[:], vrow[:])          # rs
                arow = smp.tile([1, H], f32, tag="arow")
                nc.vector.tensor_mul(arow[:], vrow[:], gb_s[:, l * H:(l + 1) * H])   # alpha
                crow = smp.tile([1, H], f32, tag="crow")
                nc.vector.tensor_mul(crow[:], mrow[:], arow[:])
                nc.vector.tensor_sub(crow[:], gb_s[:, 256 + l * H:256 + (l + 1) * H], crow[:])  # c2
                bcp = psS.tile([128, 2 * H], f32, tag="bcps")
                nc.tensor.matmul(bcp[:, 0:H], ones_s[:], arow[:], start=True, stop=True)
                nc.tensor.matmul(bcp[:, H:2 * H], ones_s[:], crow[:], start=True, stop=True)
                abc = smp.tile([128, 2 * H], f32, tag="abc")
                nc.vector.tensor_copy(abc[:], bcp[:])
                # a = relu(zt*alpha + c2)  (bf16)
                nc.vector.tensor_tensor(out=acc0[:], in0=zt[:], in1=rep_free(abc[:, 0:H], B), op=Alu.mult)
                nc.vector.tensor_tensor(out=acc0[:], in0=acc0[:], in1=rep_free(abc[:, H:2 * H], B), op=Alu.add)
                a_cur = wk.tile([128, B, H], bf16, tag="a")
                nc.scalar.activation(a_cur[:], acc0[:], Act.Relu, bias=zcol[:])
                nc.vector.memset(a_cur[96:128, B - 1:B], 0.0)   # forced-zero pad slots incl 12799

                if l < 3:
                    # transpose a -> aT [H, A] bf16
                    aT_cur = wk.tile([H, A], bf16, tag="big2")
                    for b4 in range(0, B, 4):
                        nblk = min(4, B - b4)
                        tp = psB.tile([H, 4, 128], bf16, tag="trps")
                        for bb in range(nblk):
                            nc.tensor.transpose(tp[:, bb], a_cur[:, b4 + bb], ident_s[:])
                        nc.vector.tensor_copy(aT_cur[:, (b4) * 128:(b4 + nblk) * 128],
                                              tp[:, 0:nblk].rearrange("p b f -> p (b f)"))

            # ---- pooling: per-graph mean over this core's 64 graphs ----
            pc = psS.tile([GW, H + 1], f32, tag="poolcnt")
            poolp = pc[:, 0:H]
            cntp = pc[:, H:H + 1]
            for b in range(B):
                oh = smp.tile([128, H], bf16, tag="oh", bufs=4)
                nc.vector.tensor_scalar(oh[:], iota_s[:], gl_s[:, b:b + 1], None, Alu.is_equal)
                nc.tensor.matmul(poolp, oh[:], a_cur[:, b], start=(b == 0), stop=(b == B - 1))
                nc.tensor.matmul(cntp, oh[:], onescb_s[:], start=(b == 0), stop=(b == B - 1))
            sums = smp.tile([GW, H], f32, tag="sums")
            nc.vector.tensor_copy(sums[:], poolp)
            cnts = smp.tile([GW, 1], f32, tag="cnts")
            nc.vector.tensor_copy(cnts[:], cntp)
            nc.vector.tensor_scalar_max(cnts[:], cnts[:], 1.0)
            nc.vector.reciprocal(cnts[:], cnts[:])
            nc.vector.tensor_scalar(sums[:], sums[:], cnts[:], None, Alu.mult)
            # pooledT
            ptp = psS.tile([H, GW], f32, tag="stps")
            identf = smp.tile([128, 128], f32, tag="identf")
            nc.vector.tensor_copy(identf[:], ident_s[:])
            nc.tensor.transpose(ptp[:], sums[:], identf[0:GW, 0:GW])
            pooledT = smp.tile([H, GW], f32, tag="pooledT")
            nc.vector.tensor_copy(pooledT[:], ptp[:])
            # logits = fcb + pooled @ fcW
            lgp = psS.tile([GW, C], f32, tag="bcps")
            nc.tensor.matmul(lgp[:], ones_s[:, 0:GW], fcb_s[:], start=True, stop=False)
            nc.tensor.matmul(lgp[:], pooledT[:], fcw_s[:], start=False, stop=True)
            logits = smp.tile([GW, C], f32, tag="logits")
            nc.vector.tensor_copy(logits[:], lgp[:])
            mx = smp.tile([GW, 1], f32, tag="mx")
            nc.vector.tensor_reduce(mx[:], logits[:], axis=mybir.AxisListType.X, op=Alu.max)
            nmx = smp.tile([GW, 1], f32, tag="nmx")
            nc.vector.tensor_scalar_mul(nmx[:], mx[:], -1.0)
            et = smp.tile([GW, C], f32, tag="et")
            sume = smp.tile([GW, 1], f32, tag="sume")
            nc.scalar.activation(et[:], logits[:], Act.Exp, bias=nmx[:], accum_out=sume[:])
            lse = smp.tile([GW, 1], f32, tag="lse")
            nc.scalar.activation(lse[:], sume[:], Act.Ln, bias=zcol[0:GW, :])
            res = smp.tile([GW, C], f32, tag="res")
            nc.vector.tensor_scalar(res[:], logits[:], mx[:], lse[:], Alu.subtract, Alu.subtract)
            nc.sync.dma_start(t_out[:], res[:])

    nc.compile()
    return nc


def kernel(x, edge_index, batch, W1, b1, g1, bt1, W2, b2, g2, bt2,
           W3, b3, g3, bt3, W4, b4, g4, bt4, fcW, fcb, **_unused):
    global LAST_EXEC_NS
    hp = _host_prepare(x, edge_index, batch)
    nc = _build_program(hp["addruns"])

    gb = np.zeros((1, 512), np.float32)
    for i, (g, bt) in enumerate(((g1, bt1), (g2, bt2), (g3, bt3), (g4, bt4))):
        gb[0, i * H:(i + 1) * H] = np.asarray(g, np.float32)
        gb[0, 256 + i * H:256 + (i + 1) * H] = np.asarray(bt, np.float32)
    w234 = np.concatenate([np.asarray(w, np.float32) for w in (W2, W3, W4)], axis=1)
    iota = np.tile(np.arange(H, dtype=np.float32)[None, :], (128, 1))
    common = {
        "w1": np.asarray(W1, np.float32).astype(np.dtype("bfloat16") if False else np.float32),
        "gb": gb, "fcw": np.asarray(fcW, np.float32), "fcb": np.asarray(fcb, np.float32).reshape(1, C),
        "iota": iota,
        "ones": np.ones((1, 128), np.float32),
        "onesc": np.ones((128, 1), np.float32),
        "ident": np.eye(128, dtype=np.float32),
    }
    import ml_dtypes
    bfl = ml_dtypes.bfloat16
    in_maps = []
    for k in range(NCORES):
        m = {
            "xT": hp["xT"][k].astype(bfl),
            "w1": np.asarray(W1, np.float32).astype(bfl),
            "w234": w234.astype(bfl),
            "gb": gb, "fcw": common["fcw"], "fcb": common["fcb"],
            "deg": hp["deg_loc"][k], "gl": hp["g_loc"][k],
            "iota": iota, "ones": common["ones"], "onesc": common["onesc"],
            "onescb": np.ones((128, 1), bfl),
            "ident": np.eye(128, dtype=np.float32).astype(bfl),
            "idx": hp["idx_wrapped"][k], "midx": hp["merge_idx"][k],
        }
        in_maps.append(m)

    res = run_bass_kernel_spmd(nc, in_maps, core_ids=list(range(NCORES)),
                               trace=os.environ.get("GCN_TRACE", "0") == "1")
    LAST_EXEC_NS = res.exec_time_ns
    out = np.concatenate([res.results[k]["out"] for k in range(NCORES)], axis=0)
    return out.astype(np.float32)


# revision 10
# speedup vs baseline: 2.4051x; 1.0510x over previous
"""GCN (4-layer GCNConv + BN/ReLU + mean-pool + FC + log_softmax) on 8 Trainium2 NeuronCores.

Sharding: nodes partitioned into 8 cores by contiguous 64-graph windows (graph parallel);
edges partitioned by destination core. Per layer: local matmul -> AllGather of the
dis-scaled feature table -> dma_gather edge aggregation into 4 per-source-chunk
accumulators (ELL-style pass schedule, per-chunk degree-sorted slot orderings) ->
merge -> BN (stats AllReduce) + ReLU. Final: one-hot matmul pooling + FC + log_softmax.
"""
import sys, types, os
import numpy as np


def _install_axon_hooks():
    if "antenv.axon_hooks" in sys.modules:
        return
    try:
        import antenv
    except ImportError:
        return
    mod = types.ModuleType("antenv.axon_hooks")
    state = {"hook": None}
    mod.set_axon_ntff_profile_hook = lambda h: state.__setitem__("hook", h)
    mod.get_axon_ntff_profile_hook = lambda: state["hook"]
    sys.modules["antenv.axon_hooks"] = mod
    antenv.axon_hooks = mod
    try:
        from trn_agent_boot.trn_boot import _ntff_profile_via_ctypes
        state["hook"] = _ntff_profile_via_ctypes("/opt/axon/libaxon_pjrt.so")
    except Exception:
        pass


_install_axon_hooks()

import concourse.bacc as bacc
import concourse.bass as bass
import concourse.mybir as mybir
import concourse.tile as tile
from concourse.ap import AP
from concourse.library_config import mlp
from concourse.bass_utils import run_bass_kernel_spmd

# ---- static problem shapes ----
N = 100000
E = 1600000
G = 512
FIN = 128
H = 64
C = 10
EPS = 1e-5
NCORES = 8
A = 12800            # slots per core (100 blocks of 128)
B = A // 128         # 100 blocks
GW = G // NCORES     # 64 graphs per core
NCHUNK = 4           # source chunks (pairs of cores), 25600 rows each
CHROWS = 2 * A       # rows per source chunk
CALL = 1024          # idxs per dma_gather call (single_packet limit)
NCALLS = 52          # calls per chunk per layer (52*1024 = 53248 >= max padded rows)
MCALLS = 13          # merge gather calls (13*1024 >= 12800)
DUMMY = CHROWS - 1   # in-chunk dummy row index (forced-zero slot 12799 of 2nd core)
PAD_DEG = 1.0e38

LAST_EXEC_NS = None

f32 = mybir.dt.float32
bf16 = mybir.dt.bfloat16
i16 = mybir.dt.int16
Alu = mybir.AluOpType
Act = mybir.ActivationFunctionType


def _wrap_idx(v):
    """int16 idx vector (len mult of 16) -> [128, len/16] wrapped+replicated layout."""
    blk = v.reshape(-1, 16).T.astype(np.int16)
    return np.tile(blk, (8, 1))


def _host_prepare(x, edge_index, batch):
    src = np.asarray(edge_index[0], np.int64)
    dst = np.asarray(edge_index[1], np.int64)
    batch = np.asarray(batch, np.int64)
    gsize = np.bincount(batch, minlength=G)
    gw_nodes = gsize.reshape(NCORES, GW).sum(1)
    assert gw_nodes.max() <= A, f"core node count {gw_nodes.max()} exceeds {A} slots"
    node_off = np.concatenate([[0], np.cumsum(gw_nodes)])
    core_of_node = np.repeat(np.arange(NCORES), gw_nodes)

    dst_core = core_of_node[dst]
    src_chunk = core_of_node[src] // 2

    # per (core, chunk) multiplicity of each node
    cnt = np.zeros((NCHUNK, N), np.int64)
    for q in range(NCHUNK):
        m = src_chunk == q
        cnt[q] += np.bincount(dst[m], minlength=N)
    cnt_tot = cnt.sum(0)

    # orderings: primary = chunk0-sorted; slotq = chunk-q sorted (per core)
    slot_p = np.full(N, -1, np.int64)      # node -> primary slot (0..A)
    node_of_slot = np.full((NCORES, A), -1, np.int64)
    slot_q = np.full((NCHUNK, N), -1, np.int64)
    order_q_all = {}
    for k in range(NCORES):
        nodes_k = np.arange(node_off[k], node_off[k + 1])
        for q in range(NCHUNK):
            order = nodes_k[np.argsort(-cnt[q][nodes_k], kind="stable")]
            slot_q[q][order] = np.arange(len(order))
            order_q_all[(k, q)] = order
        slot_p[order_q_all[(k, 0)]] = np.arange(len(nodes_k))
        node_of_slot[k, :len(nodes_k)] = order_q_all[(k, 0)]

    # common pass lengths L[q][j] (max over cores, 128-aligned)
    npass = np.zeros(NCHUNK, np.int64)
    nact = np.zeros((NCORES, NCHUNK, 64), np.int64)
    for k in range(NCORES):
        for q in range(NCHUNK):
            nodes_k = np.arange(node_off[k], node_off[k + 1])
            cq = cnt[q][nodes_k]
            mx = int(cq.max()) if len(cq) else 0
            npass[q] = max(npass[q], mx)
            for j in range(mx):
                nact[k, q, j] = int((cq > j).sum())
    Lpad = [[int(np.ceil(nact[:, q, j].max() / 128) * 128) for j in range(npass[q])]
            for q in range(NCHUNK)]
    for q in range(NCHUNK):
        assert sum(Lpad[q]) <= NCALLS * CALL, (q, sum(Lpad[q]))

    # gather index arrays [NCHUNK, NCALLS, 128, CALL//16] + add schedule
    idx_all = np.zeros((NCORES, NCHUNK, NCALLS * CALL), np.int16)
    idx_all[:] = DUMMY
    # table row of a source node, within its chunk: (core%2)*A + primary slot
    row_in_chunk = (core_of_node % 2) * A + slot_p
    sched = [[] for _ in range(NCHUNK)]  # per chunk: list of (pos_blk, acc_blk, nblk) common
    for q in range(NCHUNK):
        pos = 0
        for j, L in enumerate(Lpad[q]):
            sched[q].append((pos // 128, 0, L // 128, j))
            pos += L
    for k in range(NCORES):
        ek = dst_core == k
        s_k, d_k = src[ek], dst[ek]
        cq_k = src_chunk[ek]
        for q in range(NCHUNK):
            m = cq_k == q
            s_q, d_q = s_k[m], d_k[m]
            dsl = slot_q[q][d_q]
            o = np.argsort(dsl, kind="stable")
            s_q, dsl = s_q[o], dsl[o]
            # j-th edge of each slot: rank within equal dsl run
            jrank = np.arange(len(dsl)) - np.searchsorted(dsl, dsl)
            pos0 = np.concatenate([[0], np.cumsum([L for L in Lpad[q]])])
            flat = pos0[jrank] + dsl
            idx_all[k, q, flat] = row_in_chunk[s_q].astype(np.int16)

    idx_wrapped = np.zeros((NCORES, NCHUNK, NCALLS, 128, CALL // 16), np.int16)
    for k in range(NCORES):
        for q in range(NCHUNK):
            for c in range(NCALLS):
                idx_wrapped[k, q, c] = _wrap_idx(idx_all[k, q, c * CALL:(c + 1) * CALL])

    # per-call add schedule (common): call c covers staging blocks -> acc block ranges
    addsched = [[[] for _ in range(NCALLS)] for _ in range(NCHUNK)]
    for q in range(NCHUNK):
        pos = 0
        for j, L in enumerate(Lpad[q]):
            for blk in range(L // 128):
                g_abs = pos // 128 + blk
                addsched[q][g_abs // 8].append((g_abs % 8, blk))
            pos += L
    # compress consecutive runs: list of (st_blk0, acc_blk0, n)
    addruns = [[[] for _ in range(NCALLS)] for _ in range(NCHUNK)]
    for q in range(NCHUNK):
        for c in range(NCALLS):
            for st_b, ac_b in addsched[q][c]:
                runs = addruns[q][c]
                if runs and runs[-1][0] + runs[-1][2] == st_b and runs[-1][1] + runs[-1][2] == ac_b:
                    runs[-1][2] += 1
                else:
                    runs.append([st_b, ac_b, 1])

    # merge permutation: z[primary slot s] += acc_q[slot_q of node at s]
    merge_idx = np.zeros((NCORES, NCHUNK - 1, MCALLS, 128, CALL // 16), np.int16)
    for k in range(NCORES):
        nk = int(gw_nodes[k])
        for q in range(1, NCHUNK):
            mi = np.arange(A, dtype=np.int64)
            mi[:nk] = slot_q[q][node_of_slot[k, :nk]]
            mi = np.concatenate([mi, np.zeros(MCALLS * CALL - A, np.int64)])
            for c in range(MCALLS):
                merge_idx[k, q - 1, c] = _wrap_idx(mi[c * CALL:(c + 1) * CALL].astype(np.int16))

    # per-core per-slot data
    xT = np.zeros((NCORES, FIN, A), np.float32)
    deg_loc = np.full((NCORES, 128, B), PAD_DEG, np.float32)
    g_loc = np.full((NCORES, 128, B), -1.0, np.float32)
    for k in range(NCORES):
        nk = int(gw_nodes[k])
        nodes = node_of_slot[k, :nk]
        sl = np.arange(nk)
        xT[k][:, sl] = np.asarray(x, np.float32)[nodes].T
        p, bb = sl % 128, sl // 128
        deg_loc[k][p, bb] = cnt_tot[nodes].astype(np.float32)
        g_loc[k][p, bb] = (batch[nodes] - k * GW).astype(np.float32)

    return dict(gw_nodes=gw_nodes, node_off=node_off, idx_wrapped=idx_wrapped,
                addruns=addruns, merge_idx=merge_idx, xT=xT, deg_loc=deg_loc,
                g_loc=g_loc, Lpad=Lpad)


def _build_program(addruns):
    nc = bacc.Bacc("TRN2", target_bir_lowering=False, debug=False,
                   num_devices=NCORES, num_swdge_queues=4)

    # inputs
    t_xT = nc.dram_tensor("xT", [FIN, A], bf16, kind="ExternalInput")
    t_w1 = nc.dram_tensor("w1", [FIN, H], bf16, kind="ExternalInput")
    t_w = nc.dram_tensor("w234", [H, 3 * H], bf16, kind="ExternalInput")
    t_gb = nc.dram_tensor("gb", [1, 512], f32, kind="ExternalInput")
    t_fcw = nc.dram_tensor("fcw", [H, C], f32, kind="ExternalInput")
    t_fcb = nc.dram_tensor("fcb", [1, C], f32, kind="ExternalInput")
    t_deg = nc.dram_tensor("deg", [128, B], f32, kind="ExternalInput")
    t_gl = nc.dram_tensor("gl", [128, B], f32, kind="ExternalInput")
    t_iota = nc.dram_tensor("iota", [128, H], f32, kind="ExternalInput")
    t_ones = nc.dram_tensor("ones", [1, 128], f32, kind="ExternalInput")
    t_onesc = nc.dram_tensor("onesc", [128, 1], f32, kind="ExternalInput")
    t_onescb = nc.dram_tensor("onescb", [128, 1], bf16, kind="ExternalInput")
    t_ident = nc.dram_tensor("ident", [128, 128], bf16, kind="ExternalInput")
    t_idx = nc.dram_tensor("idx", [NCHUNK, NCALLS, 128, CALL // 16], i16, kind="ExternalInput")
    t_midx = nc.dram_tensor("midx", [NCHUNK - 1, MCALLS, 128, CALL // 16], i16, kind="ExternalInput")
    t_out = nc.dram_tensor("out", [GW, C], f32, kind="ExternalOutput")

    with tile.TileContext(nc) as tc:
        with tc.tile_pool(name="const", bufs=1) as cst, \
             tc.tile_pool(name="accp", bufs=1) as accp, \
             tc.tile_pool(name="work", bufs=1) as wk, \
             tc.tile_pool(name="stage", bufs=12) as stp, \
             tc.tile_pool(name="idxp", bufs=12) as idp, \
             tc.tile_pool(name="small", bufs=2) as smp, \
             tc.tile_pool(name="psA", bufs=2, space="PSUM") as psA, \
             tc.tile_pool(name="psB", bufs=2, space="PSUM") as psB, \
             tc.tile_pool(name="psS", bufs=1, space="PSUM") as psS, \
             tc.tile_pool(name="dram", bufs=1, space="DRAM") as drp:

            nc.gpsimd.load_library(mlp)

            # constants to SBUF
            xT_s = wk.tile([FIN, A], bf16, tag="big2")
            nc.sync.dma_start(xT_s[:], t_xT[:])
            w1_s = cst.tile([FIN, H], bf16)
            nc.sync.dma_start(w1_s[:], t_w1[:])
            w_s = cst.tile([H, 3 * H], bf16)
            nc.sync.dma_start(w_s[:], t_w[:])
            gb_s = cst.tile([1, 512], f32)
            nc.sync.dma_start(gb_s[:], t_gb[:])
            fcw_s = cst.tile([H, C], f32)
            nc.sync.dma_start(fcw_s[:], t_fcw[:])
            fcb_s = cst.tile([1, C], f32)
            nc.sync.dma_start(fcb_s[:], t_fcb[:])
            deg_s = cst.tile([128, B], f32)
            nc.sync.dma_start(deg_s[:], t_deg[:])
            gl_s = cst.tile([128, B], f32)
            nc.sync.dma_start(gl_s[:], t_gl[:])
            iota_s = cst.tile([128, H], f32)
            nc.sync.dma_start(iota_s[:], t_iota[:])
            ones_s = cst.tile([1, 128], f32)
            nc.sync.dma_start(ones_s[:], t_ones[:])
            onesc_s = cst.tile([128, 1], f32)
            nc.sync.dma_start(onesc_s[:], t_onesc[:])
            onescb_s = cst.tile([128, 1], bf16)
            nc.sync.dma_start(onescb_s[:], t_onescb[:])
            ident_s = cst.tile([128, 128], bf16)
            nc.sync.dma_start(ident_s[:], t_ident[:])

            zcol = cst.tile([128, 1], f32)
            nc.vector.memset(zcol[:], 0.0)
            epst = cst.tile([1, 1], f32)
            nc.vector.memset(epst[:], EPS)
            # dis = 1/sqrt(deg+1)
            dis_s = cst.tile([128, B], f32)
            nc.scalar.activation(dis_s[:], deg_s[:], Act.Sqrt, bias=onesc_s[:])
            nc.vector.reciprocal(dis_s[:], dis_s[:])

            def bcast_dis(bsl):  # dis slice [128, nb] -> AP [128, nb, 64]
                s = dis_s[:, bsl]
                return AP(s.tensor, s.offset, [s.ap[0], s.ap[1], [0, H]])

            def rep_free(ap2d, n):  # [P, F] -> [P, n, F] (free repeat)
                return AP(ap2d.tensor, ap2d.offset, [ap2d.ap[0], [0, n], ap2d.ap[1]])

            a_cur = None       # [128, B, H] bf16 post-BN activation (node-major slots)
            aT_cur = None      # [H, A] bf16 transposed

            for l in range(4):
                # ---- local matmul: h_scaled_local = (a @ W) * dis ----
                acc0 = accp.tile([128, B, H], f32, tag="acc0")
                K = FIN if l == 0 else H
                lhsT_full = xT_s if l == 0 else aT_cur
                W_ap = w1_s[:] if l == 0 else w_s[:, (l - 1) * H:l * H]
                for b8 in range(0, B, 8):
                    nblk = min(8, B - b8)
                    pt = psA.tile([128, 8, H], f32, tag="mmps")
                    for bb in range(nblk):
                        nc.tensor.matmul(pt[:, bb], lhsT_full[:, (b8 + bb) * 128:(b8 + bb + 1) * 128],
                                         W_ap, start=True, stop=True)
                    nc.vector.tensor_tensor(out=acc0[:, b8:b8 + nblk], in0=pt[:, 0:nblk],
                                            in1=bcast_dis(slice(b8, b8 + nblk)), op=Alu.mult)
                # shard -> DRAM (row-major by slot: row s=(p + 128*b))
                shard = drp.tile([A, H], f32, tag="shard")
                sh_ap = AP(shard[:].tensor, shard[:].offset,
                           [[H, 128], [128 * H, B], [1, H]])
                nc.sync.dma_start(sh_ap, acc0[:])
                table = drp.tile([NCORES * A, H], f32, tag="table", addr_space="Shared")
                nc.gpsimd.collective_compute(
                    "AllGather", Alu.bypass,
                    replica_groups=[list(range(NCORES))],
                    ins=[shard[:]], outs=[table[:]])

                # ---- edge gathers into 4 accumulators ----
                accq = [acc0]
                for q in range(1, NCHUNK):
                    aq = accp.tile([128, B, H], f32, tag=f"acc{q}")
                    nc.vector.memset(aq[:], 0.0)
                    accq.append(aq)
                for c in range(NCALLS):
                    for q in range(NCHUNK):
                        if not addruns[q][c]:
                            continue
                        src_ap = table[q * CHROWS:(q + 1) * CHROWS, :]
                        it = idp.tile([128, CALL // 16], i16, tag="idx")
                        nc.sync.dma_start(it[:], t_idx[q, c])
                        st = stp.tile([128, 8, H], f32, tag="stage")
                        nc.gpsimd.dma_gather(st[:], src_ap, it[:], CALL, CALL, H,
                                             single_packet=True, queue_num=q)
                        for st_b, ac_b, nb in addruns[q][c]:
                            nc.vector.tensor_add(accq[q][:, ac_b:ac_b + nb],
                                                 accq[q][:, ac_b:ac_b + nb],
                                                 st[:, st_b:st_b + nb])
                # ---- merge acc1..3 into acc0 (permuted via local gather) ----
                scrs = {}
                for q in range(1, NCHUNK):
                    scr = drp.tile([A, H], f32, tag=f"scr{q}")
                    sc_ap = AP(scr[:].tensor, scr[:].offset,
                               [[H, 128], [128 * H, B], [1, H]])
                    nc.sync.dma_start(sc_ap, accq[q][:])
                    scrs[q] = scr
                for c in range(MCALLS):
                    for q in range(1, NCHUNK):
                        nblk = min(8, B - c * 8)
                        it = idp.tile([128, CALL // 16], i16, tag="idx")
                        nc.sync.dma_start(it[:], t_midx[q - 1, c])
                        st = stp.tile([128, 8, H], f32, tag="stage")
                        nc.gpsimd.dma_gather(st[:], scrs[q][:], it[:], CALL, CALL, H,
                                             single_packet=True, queue_num=(q + c) % 4)
                        nc.vector.tensor_add(acc0[:, c * 8:c * 8 + nblk],
                                             acc0[:, c * 8:c * 8 + nblk], st[:, 0:nblk])

                # ---- zt = acc0 * dis ; stats; BN+ReLU ----
                nc.vector.tensor_tensor(out=acc0[:], in0=acc0[:], in1=bcast_dis(slice(0, B)), op=Alu.mult)
                zt = acc0
                s1t = smp.tile([128, H], f32, tag="s1")
                nc.vector.tensor_reduce(s1t[:], zt[:].rearrange("p b f -> p f b"),
                                        axis=mybir.AxisListType.X, op=Alu.add)
                sq = wk.tile([128, B, H], f32, tag="big2")
                nc.vector.tensor_mul(sq[:], zt[:], zt[:])
                s2t = smp.tile([128, H], f32, tag="s2")
                nc.vector.tensor_reduce(s2t[:], sq[:].rearrange("p b f -> p f b"),
                                        axis=mybir.AxisListType.X, op=Alu.add)
                spt = psS.tile([1, 128], f32, tag="stps")
                nc.tensor.matmul(spt[:, 0:H], onesc_s[:], s1t[:], start=True, stop=True)
                nc.tensor.matmul(spt[:, H:2 * H], onesc_s[:], s2t[:], start=True, stop=True)
                srow = smp.tile([1, 128], f32, tag="srow")
                nc.vector.tensor_copy(srow[:], spt[:])
                arb_in = drp.tile([1, 128], f32, tag="arbin")
                arb_out = drp.tile([1, 128], f32, tag="arbout", addr_space="Shared")
                nc.sync.dma_start(arb_in[:], srow[:])
                nc.gpsimd.collective_compute(
                    "AllReduce", Alu.add,
                    replica_groups=[list(range(NCORES))],
                    ins=[arb_in[:]], outs=[arb_out[:]])
                sg = smp.tile([1, 128], f32, tag="sg")
                nc.sync.dma_start(sg[:], arb_out[:])
                # m=sg[0:64]/N; ex2=sg[64:128]/N; var=ex2-m*m; rs=1/sqrt(var+eps)
                mrow = smp.tile([1, H], f32, tag="mrow")
                nc.vector.tensor_scalar_mul(mrow[:], sg[:, 0:H], 1.0 / N)
                vrow = smp.tile([1, H], f32, tag="vrow")
                nc.vector.tensor_scalar_mul(vrow[:], sg[:, H:2 * H], 1.0 / N)
                mm = smp.tile([1, H], f32, tag="mm")
                nc.vector.tensor_mul(mm[:], mrow[:], mrow[:])
                nc.vector.tensor_sub(vrow[:], vrow[:], mm[:])
                nc.scalar.activation(vrow[:], vrow[:], Act.Sqrt, bias=epst[:])
                nc.vector.reciprocal(vrow[:], vrow[:])          # rs
                arow = smp.tile([1, H], f32, tag="arow")
                nc.vector.tensor_mul(arow[:], vrow[:], gb_s[:, l * H:(l + 1) * H])   # alpha
                crow = smp.tile([1, H], f32, tag="crow")
                nc.vector.tensor_mul(crow[:], mrow[:], arow[:])
                nc.vector.tensor_sub(crow[:], gb_s[:, 256 + l * H:256 + (l + 1) * H], crow[:])  # c2
                bcp = psS.tile([128, 2 * H], f32, tag="bcps")
                nc.tensor.matmul(bcp[:, 0:H], ones_s[:], arow[:], start=True, stop=True)
                nc.tensor.matmul(bcp[:, H:2 * H], ones_s[:], crow[:], start=True, stop=True)
                abc = smp.tile([128, 2 * H], f32, tag="abc")
                nc.vector.tensor_copy(abc[:], bcp[:])
                # a = relu(zt*alpha + c2)  (bf16)
                nc.vector.tensor_tensor(out=acc0[:], in0=zt[:], in1=rep_free(abc[:, 0:H], B), op=Alu.mult)
                nc.vector.tensor_tensor(out=acc0[:], in0=acc0[:], in1=rep_free(abc[:, H:2 * H], B), op=Alu.add)
                a_cur = wk.tile([128, B, H], bf16, tag="a")
                nc.scalar.activation(a_cur[:], acc0[:], Act.Relu, bias=zcol[:])
                nc.vector.memset(a_cur[96:128, B - 1:B], 0.0)   # forced-zero pad slots incl 12799

                if l < 3:
                    # transpose a -> aT [H, A] bf16
                    aT_cur = wk.tile([H, A], bf16, tag="big2")
                    for b4 in range(0, B, 4):
                        nblk = min(4, B - b4)
                        tp = psB.tile([H, 4, 128], bf16, tag="trps")
                        for bb in range(nblk):
                            nc.tensor.transpose(tp[:, bb], a_cur[:, b4 + bb], ident_s[:])
                        nc.vector.tensor_copy(aT_cur[:, (b4) * 128:(b4 + nblk) * 128],
                                              tp[:, 0:nblk].rearrange("p b f -> p (b f)"))

            # ---- pooling: per-graph mean over this core's 64 graphs ----
            pc = psS.tile([GW, H + 1], f32, tag="poolcnt")
            poolp = pc[:, 0:H]
            cntp = pc[:, H:H + 1]
            for b in range(B):
                oh = smp.tile([128, H], bf16, tag="oh", bufs=4)
                nc.vector.tensor_scalar(oh[:], iota_s[:], gl_s[:, b:b + 1], None, Alu.is_equal)
                nc.tensor.matmul(poolp, oh[:], a_cur[:, b], start=(b == 0), stop=(b == B - 1))
                nc.tensor.matmul(cntp, oh[:], onescb_s[:], start=(b == 0), stop=(b == B - 1))
            sums = smp.tile([GW, H], f32, tag="sums")
            nc.vector.tensor_copy(sums[:], poolp)
            cnts = smp.tile([GW, 1], f32, tag="cnts")
            nc.vector.tensor_copy(cnts[:], cntp)
            nc.vector.tensor_scalar_max(cnts[:], cnts[:], 1.0)
            nc.vector.reciprocal(cnts[:], cnts[:])
            nc.vector.tensor_scalar(sums[:], sums[:], cnts[:], None, Alu.mult)
            # pooledT
            ptp = psS.tile([H, GW], f32, tag="stps")
            identf = smp.tile([128, 128], f32, tag="identf")
            nc.vector.tensor_copy(identf[:], ident_s[:])
            nc.tensor.transpose(ptp[:], sums[:], identf[0:GW, 0:GW])
            pooledT = smp.tile([H, GW], f32, tag="pooledT")
            nc.vector.tensor_copy(pooledT[:], ptp[:])
            # logits = fcb + pooled @ fcW
            lgp = psS.tile([GW, C], f32, tag="bcps")
            nc.tensor.matmul(lgp[:], ones_s[:, 0:GW], fcb_s[:], start=True, stop=False)
            nc.tensor.matmul(lgp[:], pooledT[:], fcw_s[:], start=False, stop=True)
            logits = smp.tile([GW, C], f32, tag="logits")
            nc.vector.tensor_copy(logits[:], lgp[:])
            mx = smp.tile([GW, 1], f32, tag="mx")
            nc.vector.tensor_reduce(mx[:], logits[:], axis=mybir.AxisListType.X, op=Alu.max)
            nmx = smp.tile([GW, 1], f32, tag="nmx")
            nc.vector.tensor_scalar_mul(nmx[:], mx[:], -1.0)
            et = smp.tile([GW, C], f32, tag="et")
            sume = smp.tile([GW, 1], f32, tag="sume")
            nc.scalar.activation(et[:], logits[:], Act.Exp, bias=nmx[:], accum_out=sume[:])
            lse = smp.tile([GW, 1], f32, tag="lse")
            nc.scalar.activation(lse[:], sume[:], Act.Ln, bias=zcol[0:GW, :])
            res = smp.tile([GW, C], f32, tag="res")
            nc.vector.tensor_scalar(res[:], logits[:], mx[:], lse[:], Alu.subtract, Alu.subtract)
            nc.sync.dma_start(t_out[:], res[:])

    nc.compile()
    return nc


def kernel(x, edge_index, batch, W1, b1, g1, bt1, W2, b2, g2, bt2,
           W3, b3, g3, bt3, W4, b4, g4, bt4, fcW, fcb, **_unused):
    global LAST_EXEC_NS
    hp = _host_prepare(x, edge_index, batch)
    nc = _build_program(hp["addruns"])

    gb = np.zeros((1, 512), np.float32)
    for i, (g, bt) in enumerate(((g1, bt1), (g2, bt2), (g3, bt3), (g4, bt4))):
        gb[0, i * H:(i + 1) * H] = np.asarray(g, np.float32)
        gb[0, 256 + i * H:256 + (i + 1) * H] = np.asarray(bt, np.float32)
    w234 = np.concatenate([np.asarray(w, np.float32) for w in (W2, W3, W4)], axis=1)
    iota = np.tile(np.arange(H, dtype=np.float32)[None, :], (128, 1))
    common = {
        "w1": np.asarray(W1, np.float32).astype(np.dtype("bfloat16") if False else np.float32),
        "gb": gb, "fcw": np.asarray(fcW, np.float32), "fcb": np.asarray(fcb, np.float32).reshape(1, C),
        "iota": iota,
        "ones": np.ones((1, 128), np.float32),
        "onesc": np.ones((128, 1), np.float32),
        "ident": np.eye(128, dtype=np.float32),
    }
    import ml_dtypes
    bfl = ml_dtypes.bfloat16
    in_maps = []
    for k in range(NCORES):
        m = {
            "xT": hp["xT"][k].astype(bfl),
            "w1": np.asarray(W1, np.float32).astype(bfl),
            "w234": w234.astype(bfl),
            "gb": gb, "fcw": common["fcw"], "fcb": common["fcb"],
            "deg": hp["deg_loc"][k], "gl": hp["g_loc"][k],
            "iota": iota, "ones": common["ones"], "onesc": common["onesc"],
            "onescb": np.ones((128, 1), bfl),
            "ident": np.eye(128, dtype=np.float32).astype(bfl),
            "idx": hp["idx_wrapped"][k], "midx": hp["merge_idx"][k],
        }
        in_maps.append(m)

    res = run_bass_kernel_spmd(nc, in_maps, core_ids=list(range(NCORES)),
                               trace=os.environ.get("GCN_TRACE", "0") == "1")
    LAST_EXEC_NS = res.exec_time_ns
    out = np.concatenate([res.results[k]["out"] for k in range(NCORES)], axis=0)
    return out.astype(np.float32)


# revision 12
# speedup vs baseline: 2.4344x; 1.0122x over previous
"""GCN (4-layer GCNConv + BN/ReLU + mean-pool + FC + log_softmax) on 8 Trainium2 NeuronCores.

Sharding: nodes partitioned into 8 cores by contiguous 64-graph windows (graph parallel);
edges partitioned by destination core. Per layer: local matmul -> AllGather of the
dis-scaled feature table -> dma_gather edge aggregation into 4 per-source-chunk
accumulators (ELL-style pass schedule, per-chunk degree-sorted slot orderings) ->
merge -> BN (stats AllReduce) + ReLU. Final: one-hot matmul pooling + FC + log_softmax.
"""
import sys, types, os
import numpy as np


def _install_axon_hooks():
    if "antenv.axon_hooks" in sys.modules:
        return
    try:
        import antenv
    except ImportError:
        return
    mod = types.ModuleType("antenv.axon_hooks")
    state = {"hook": None}
    mod.set_axon_ntff_profile_hook = lambda h: state.__setitem__("hook", h)
    mod.get_axon_ntff_profile_hook = lambda: state["hook"]
    sys.modules["antenv.axon_hooks"] = mod
    antenv.axon_hooks = mod
    try:
        from trn_agent_boot.trn_boot import _ntff_profile_via_ctypes
        state["hook"] = _ntff_profile_via_ctypes("/opt/axon/libaxon_pjrt.so")
    except Exception:
        pass


_install_axon_hooks()

import concourse.bacc as bacc
import concourse.bass as bass
import concourse.mybir as mybir
import concourse.tile as tile
from concourse.ap import AP
from concourse.library_config import mlp
from concourse.bass_utils import run_bass_kernel_spmd

# ---- static problem shapes ----
N = 100000
E = 1600000
G = 512
FIN = 128
H = 64
C = 10
EPS = 1e-5
NCORES = 8
A = 12800            # slots per core (100 blocks of 128)
B = A // 128         # 100 blocks
GW = G // NCORES     # 64 graphs per core
NCHUNK = 4           # source chunks (pairs of cores), 25600 rows each
CHROWS = 2 * A       # rows per source chunk
CALL = 1024          # idxs per dma_gather call (single_packet limit)
NCALLS = 52          # calls per chunk per layer (52*1024 = 53248 >= max padded rows)
MCALLS = 13          # merge gather calls (13*1024 >= 12800)
DUMMY = CHROWS - 1   # in-chunk dummy row index (forced-zero slot 12799 of 2nd core)
PAD_DEG = 1.0e38

LAST_EXEC_NS = None

f32 = mybir.dt.float32
bf16 = mybir.dt.bfloat16
i16 = mybir.dt.int16
Alu = mybir.AluOpType
Act = mybir.ActivationFunctionType


def _wrap_idx(v):
    """int16 idx vector (len mult of 16) -> [128, len/16] wrapped+replicated layout."""
    blk = v.reshape(-1, 16).T.astype(np.int16)
    return np.tile(blk, (8, 1))


def _host_prepare(x, edge_index, batch):
    src = np.asarray(edge_index[0], np.int64)
    dst = np.asarray(edge_index[1], np.int64)
    batch = np.asarray(batch, np.int64)
    gsize = np.bincount(batch, minlength=G)
    gw_nodes = gsize.reshape(NCORES, GW).sum(1)
    assert gw_nodes.max() <= A, f"core node count {gw_nodes.max()} exceeds {A} slots"
    node_off = np.concatenate([[0], np.cumsum(gw_nodes)])
    core_of_node = np.repeat(np.arange(NCORES), gw_nodes)

    dst_core = core_of_node[dst]
    src_chunk = core_of_node[src] // 2

    # per (core, chunk) multiplicity of each node
    cnt = np.zeros((NCHUNK, N), np.int64)
    for q in range(NCHUNK):
        m = src_chunk == q
        cnt[q] += np.bincount(dst[m], minlength=N)
    cnt_tot = cnt.sum(0)

    # orderings: primary = chunk0-sorted; slotq = chunk-q sorted (per core)
    slot_p = np.full(N, -1, np.int64)      # node -> primary slot (0..A)
    node_of_slot = np.full((NCORES, A), -1, np.int64)
    slot_q = np.full((NCHUNK, N), -1, np.int64)
    order_q_all = {}
    for k in range(NCORES):
        nodes_k = np.arange(node_off[k], node_off[k + 1])
        for q in range(NCHUNK):
            order = nodes_k[np.argsort(-cnt[q][nodes_k], kind="stable")]
            slot_q[q][order] = np.arange(len(order))
            order_q_all[(k, q)] = order
        slot_p[order_q_all[(k, 0)]] = np.arange(len(nodes_k))
        node_of_slot[k, :len(nodes_k)] = order_q_all[(k, 0)]

    # common pass lengths L[q][j] (max over cores, 128-aligned)
    npass = np.zeros(NCHUNK, np.int64)
    nact = np.zeros((NCORES, NCHUNK, 64), np.int64)
    for k in range(NCORES):
        for q in range(NCHUNK):
            nodes_k = np.arange(node_off[k], node_off[k + 1])
            cq = cnt[q][nodes_k]
            mx = int(cq.max()) if len(cq) else 0
            npass[q] = max(npass[q], mx)
            for j in range(mx):
                nact[k, q, j] = int((cq > j).sum())
    Lpad = [[int(np.ceil(nact[:, q, j].max() / 128) * 128) for j in range(npass[q])]
            for q in range(NCHUNK)]
    for q in range(NCHUNK):
        assert sum(Lpad[q]) <= NCALLS * CALL, (q, sum(Lpad[q]))

    # gather index arrays [NCHUNK, NCALLS, 128, CALL//16] + add schedule
    idx_all = np.zeros((NCORES, NCHUNK, NCALLS * CALL), np.int16)
    idx_all[:] = DUMMY
    # table row of a source node, within its chunk: (core%2)*A + primary slot
    row_in_chunk = (core_of_node % 2) * A + slot_p
    sched = [[] for _ in range(NCHUNK)]  # per chunk: list of (pos_blk, acc_blk, nblk) common
    for q in range(NCHUNK):
        pos = 0
        for j, L in enumerate(Lpad[q]):
            sched[q].append((pos // 128, 0, L // 128, j))
            pos += L
    for k in range(NCORES):
        ek = dst_core == k
        s_k, d_k = src[ek], dst[ek]
        cq_k = src_chunk[ek]
        for q in range(NCHUNK):
            m = cq_k == q
            s_q, d_q = s_k[m], d_k[m]
            dsl = slot_q[q][d_q]
            o = np.argsort(dsl, kind="stable")
            s_q, dsl = s_q[o], dsl[o]
            # j-th edge of each slot: rank within equal dsl run
            jrank = np.arange(len(dsl)) - np.searchsorted(dsl, dsl)
            pos0 = np.concatenate([[0], np.cumsum([L for L in Lpad[q]])])
            flat = pos0[jrank] + dsl
            idx_all[k, q, flat] = row_in_chunk[s_q].astype(np.int16)

    idx_wrapped = np.zeros((NCORES, NCHUNK, NCALLS, 128, CALL // 16), np.int16)
    for k in range(NCORES):
        for q in range(NCHUNK):
            for c in range(NCALLS):
                idx_wrapped[k, q, c] = _wrap_idx(idx_all[k, q, c * CALL:(c + 1) * CALL])

    # per-call add schedule (common): call c covers staging blocks -> acc block ranges
    addsched = [[[] for _ in range(NCALLS)] for _ in range(NCHUNK)]
    for q in range(NCHUNK):
        pos = 0
        for j, L in enumerate(Lpad[q]):
            for blk in range(L // 128):
                g_abs = pos // 128 + blk
                addsched[q][g_abs // 8].append((g_abs % 8, blk))
            pos += L
    # compress consecutive runs: list of (st_blk0, acc_blk0, n)
    addruns = [[[] for _ in range(NCALLS)] for _ in range(NCHUNK)]
    for q in range(NCHUNK):
        for c in range(NCALLS):
            for st_b, ac_b in addsched[q][c]:
                runs = addruns[q][c]
                if runs and runs[-1][0] + runs[-1][2] == st_b and runs[-1][1] + runs[-1][2] == ac_b:
                    runs[-1][2] += 1
                else:
                    runs.append([st_b, ac_b, 1])

    # merge permutation: z[primary slot s] += acc_q[slot_q of node at s]
    merge_idx = np.zeros((NCORES, NCHUNK - 1, MCALLS, 128, CALL // 16), np.int16)
    for k in range(NCORES):
        nk = int(gw_nodes[k])
        for q in range(1, NCHUNK):
            mi = np.arange(A, dtype=np.int64)
            mi[:nk] = slot_q[q][node_of_slot[k, :nk]]
            mi = np.concatenate([mi, np.zeros(MCALLS * CALL - A, np.int64)])
            for c in range(MCALLS):
                merge_idx[k, q - 1, c] = _wrap_idx(mi[c * CALL:(c + 1) * CALL].astype(np.int16))

    # per-core per-slot data
    xT = np.zeros((NCORES, FIN, A), np.float32)
    deg_loc = np.full((NCORES, 128, B), PAD_DEG, np.float32)
    g_loc = np.full((NCORES, 128, B), -1.0, np.float32)
    for k in range(NCORES):
        nk = int(gw_nodes[k])
        nodes = node_of_slot[k, :nk]
        sl = np.arange(nk)
        xT[k][:, sl] = np.asarray(x, np.float32)[nodes].T
        p, bb = sl % 128, sl // 128
        deg_loc[k][p, bb] = cnt_tot[nodes].astype(np.float32)
        g_loc[k][p, bb] = (batch[nodes] - k * GW).astype(np.float32)

    return dict(gw_nodes=gw_nodes, node_off=node_off, idx_wrapped=idx_wrapped,
                addruns=addruns, merge_idx=merge_idx, xT=xT, deg_loc=deg_loc,
                g_loc=g_loc, Lpad=Lpad)


def _build_program(addruns):
    nc = bacc.Bacc("TRN2", target_bir_lowering=False, debug=False,
                   num_devices=NCORES, num_swdge_queues=4)

    # inputs
    t_xT = nc.dram_tensor("xT", [FIN, A], bf16, kind="ExternalInput")
    t_w1 = nc.dram_tensor("w1", [FIN, H], bf16, kind="ExternalInput")
    t_w = nc.dram_tensor("w234", [H, 3 * H], bf16, kind="ExternalInput")
    t_gb = nc.dram_tensor("gb", [H, 8], f32, kind="ExternalInput")
    t_fcw = nc.dram_tensor("fcw", [H, C], f32, kind="ExternalInput")
    t_fcb = nc.dram_tensor("fcb", [1, C], f32, kind="ExternalInput")
    t_deg = nc.dram_tensor("deg", [128, B], f32, kind="ExternalInput")
    t_gl = nc.dram_tensor("gl", [128, B], f32, kind="ExternalInput")
    t_iota = nc.dram_tensor("iota", [128, H], f32, kind="ExternalInput")
    t_ones = nc.dram_tensor("ones", [1, 128], f32, kind="ExternalInput")
    t_onesc = nc.dram_tensor("onesc", [128, 1], f32, kind="ExternalInput")
    t_onescb = nc.dram_tensor("onescb", [128, 1], bf16, kind="ExternalInput")
    t_ident = nc.dram_tensor("ident", [128, 128], bf16, kind="ExternalInput")
    t_idx = nc.dram_tensor("idx", [NCHUNK, NCALLS, 128, CALL // 16], i16, kind="ExternalInput")
    t_midx = nc.dram_tensor("midx", [NCHUNK - 1, MCALLS, 128, CALL // 16], i16, kind="ExternalInput")
    t_out = nc.dram_tensor("out", [GW, C], f32, kind="ExternalOutput")

    with tile.TileContext(nc) as tc:
        with tc.tile_pool(name="const", bufs=1) as cst, \
             tc.tile_pool(name="accp", bufs=1) as accp, \
             tc.tile_pool(name="work", bufs=1) as wk, \
             tc.tile_pool(name="stage", bufs=12) as stp, \
             tc.tile_pool(name="idxp", bufs=12) as idp, \
             tc.tile_pool(name="small", bufs=2) as smp, \
             tc.tile_pool(name="psA", bufs=2, space="PSUM") as psA, \
             tc.tile_pool(name="psB", bufs=2, space="PSUM") as psB, \
             tc.tile_pool(name="psS", bufs=1, space="PSUM") as psS, \
             tc.tile_pool(name="dram", bufs=1, space="DRAM") as drp:

            nc.gpsimd.load_library(mlp)

            # constants to SBUF
            xT_s = wk.tile([FIN, A], bf16, tag="big2")
            nc.sync.dma_start(xT_s[:], t_xT[:])
            w1_s = cst.tile([FIN, H], bf16)
            nc.sync.dma_start(w1_s[:], t_w1[:])
            w_s = cst.tile([H, 3 * H], bf16)
            nc.sync.dma_start(w_s[:], t_w[:])
            gb_s = cst.tile([H, 8], f32)
            nc.sync.dma_start(gb_s[:], t_gb[:])
            fcw_s = cst.tile([H, C], f32)
            nc.sync.dma_start(fcw_s[:], t_fcw[:])
            fcb_s = cst.tile([1, C], f32)
            nc.sync.dma_start(fcb_s[:], t_fcb[:])
            deg_s = cst.tile([128, B], f32)
            nc.sync.dma_start(deg_s[:], t_deg[:])
            gl_s = cst.tile([128, B], f32)
            nc.sync.dma_start(gl_s[:], t_gl[:])
            iota_s = cst.tile([128, H], f32)
            nc.sync.dma_start(iota_s[:], t_iota[:])
            ones_s = cst.tile([1, 128], f32)
            nc.sync.dma_start(ones_s[:], t_ones[:])
            onesc_s = cst.tile([128, 1], f32)
            nc.sync.dma_start(onesc_s[:], t_onesc[:])
            onescb_s = cst.tile([128, 1], bf16)
            nc.sync.dma_start(onescb_s[:], t_onescb[:])
            ident_s = cst.tile([128, 128], bf16)
            nc.sync.dma_start(ident_s[:], t_ident[:])

            zcol = cst.tile([128, 1], f32)
            nc.vector.memset(zcol[:], 0.0)
            epsc = cst.tile([H, 1], f32)
            nc.vector.memset(epsc[:], EPS)
            identf = cst.tile([128, 128], f32)
            nc.vector.tensor_copy(identf[:], ident_s[:])
            # dis = 1/sqrt(deg+1)
            dis_s = cst.tile([128, B], f32)
            nc.scalar.activation(dis_s[:], deg_s[:], Act.Sqrt, bias=onesc_s[:])
            nc.vector.reciprocal(dis_s[:], dis_s[:])

            def bcast_dis(bsl):  # dis slice [128, nb] -> AP [128, nb, 64]
                s = dis_s[:, bsl]
                return AP(s.tensor, s.offset, [s.ap[0], s.ap[1], [0, H]])

            def rep_free(ap2d, n):  # [P, F] -> [P, n, F] (free repeat)
                return AP(ap2d.tensor, ap2d.offset, [ap2d.ap[0], [0, n], ap2d.ap[1]])

            a_cur = None       # [128, B, H] bf16 post-BN activation (node-major slots)
            aT_cur = None      # [H, A] bf16 transposed

            for l in range(4):
                # ---- local matmul: h_scaled_local = (a @ W) * dis ----
                acc0 = accp.tile([128, B, H], f32, tag="acc0")
                K = FIN if l == 0 else H
                lhsT_full = xT_s if l == 0 else aT_cur
                W_ap = w1_s[:] if l == 0 else w_s[:, (l - 1) * H:l * H]
                for b8 in range(0, B, 8):
                    nblk = min(8, B - b8)
                    pt = psA.tile([128, 8, H], f32, tag="mmps")
                    for bb in range(nblk):
                        nc.tensor.matmul(pt[:, bb], lhsT_full[:, (b8 + bb) * 128:(b8 + bb + 1) * 128],
                                         W_ap, start=True, stop=True)
                    nc.vector.tensor_tensor(out=acc0[:, b8:b8 + nblk], in0=pt[:, 0:nblk],
                                            in1=bcast_dis(slice(b8, b8 + nblk)), op=Alu.mult)
                # shard -> DRAM (row-major by slot: row s=(p + 128*b))
                shard = drp.tile([A, H], f32, tag="shard")
                sh_ap = AP(shard[:].tensor, shard[:].offset,
                           [[H, 128], [128 * H, B], [1, H]])
                nc.sync.dma_start(sh_ap, acc0[:])
                table = drp.tile([NCORES * A, H], f32, tag="table", addr_space="Shared")
                nc.gpsimd.collective_compute(
                    "AllGather", Alu.bypass,
                    replica_groups=[list(range(NCORES))],
                    ins=[shard[:]], outs=[table[:]])

                # ---- edge gathers into 4 accumulators ----
                accq = [acc0]
                for q in range(1, NCHUNK):
                    aq = accp.tile([128, B, H], f32, tag=f"acc{q}")
                    nc.vector.memset(aq[:], 0.0)
                    accq.append(aq)
                for c in range(NCALLS):
                    for q in range(NCHUNK):
                        if not addruns[q][c]:
                            continue
                        src_ap = table[q * CHROWS:(q + 1) * CHROWS, :]
                        it = idp.tile([128, CALL // 16], i16, tag="idx")
                        nc.sync.dma_start(it[:], t_idx[q, c])
                        st = stp.tile([128, 8, H], f32, tag="stage")
                        nc.gpsimd.dma_gather(st[:], src_ap, it[:], CALL, CALL, H,
                                             single_packet=True, queue_num=q)
                        for st_b, ac_b, nb in addruns[q][c]:
                            nc.vector.tensor_add(accq[q][:, ac_b:ac_b + nb],
                                                 accq[q][:, ac_b:ac_b + nb],
                                                 st[:, st_b:st_b + nb])
                # ---- merge acc1..3 into acc0 (permuted via local gather) ----
                scrs = {}
                for q in range(1, NCHUNK):
                    scr = drp.tile([A, H], f32, tag=f"scr{q}")
                    sc_ap = AP(scr[:].tensor, scr[:].offset,
                               [[H, 128], [128 * H, B], [1, H]])
                    nc.sync.dma_start(sc_ap, accq[q][:])
                    scrs[q] = scr
                for c in range(MCALLS):
                    for q in range(1, NCHUNK):
                        nblk = min(8, B - c * 8)
                        it = idp.tile([128, CALL // 16], i16, tag="idx")
                        nc.sync.dma_start(it[:], t_midx[q - 1, c])
                        st = stp.tile([128, 8, H], f32, tag="stage")
                        nc.gpsimd.dma_gather(st[:], scrs[q][:], it[:], CALL, CALL, H,
                                             single_packet=True, queue_num=(q + c) % 4)
                        nc.vector.tensor_add(acc0[:, c * 8:c * 8 + nblk],
                                             acc0[:, c * 8:c * 8 + nblk], st[:, 0:nblk])

                # ---- zt = acc0 * dis ; stats; BN+ReLU ----
                nc.vector.tensor_tensor(out=acc0[:], in0=acc0[:], in1=bcast_dis(slice(0, B)), op=Alu.mult)
                zt = acc0
                s1t = smp.tile([128, H], f32, tag="s1")
                nc.vector.tensor_reduce(s1t[:], zt[:].rearrange("p b f -> p f b"),
                                        axis=mybir.AxisListType.X, op=Alu.add)
                sq = wk.tile([128, B, H], f32, tag="big2")
                nc.vector.tensor_mul(sq[:], zt[:], zt[:])
                s2t = smp.tile([128, H], f32, tag="s2")
                nc.vector.tensor_reduce(s2t[:], sq[:].rearrange("p b f -> p f b"),
                                        axis=mybir.AxisListType.X, op=Alu.add)
                spt = psS.tile([H, 2], f32, tag="stps")
                nc.tensor.matmul(spt[:, 0:1], s1t[:], onesc_s[:], start=True, stop=True)
                nc.tensor.matmul(spt[:, 1:2], s2t[:], onesc_s[:], start=True, stop=True)
                # transpose zt -> aT_pre (bf16 f-major) BEFORE the AllReduce (overlaps AR)
                if l < 3:
                    aT_pre = wk.tile([H, A], bf16, tag="big2")
                    for b4 in range(0, B, 4):
                        nblk = min(4, B - b4)
                        tpf = psB.tile([H, 4, 128], f32, tag="trps")
                        for bb in range(nblk):
                            nc.tensor.transpose(tpf[:, bb], zt[:, b4 + bb], identf[:])
                        nc.vector.tensor_copy(aT_pre[:, b4 * 128:(b4 + nblk) * 128],
                                              tpf[:, 0:nblk].rearrange("p b f -> p (b f)"))
                scol = smp.tile([H, 2], f32, tag="scol")
                nc.vector.tensor_copy(scol[:], spt[:])
                arb_in = drp.tile([H, 2], f32, tag="arbin")
                arb_out = drp.tile([H, 2], f32, tag="arbout", addr_space="Shared")
                nc.sync.dma_start(arb_in[:], scol[:])
                nc.gpsimd.collective_compute(
                    "AllReduce", Alu.add,
                    replica_groups=[list(range(NCORES))],
                    ins=[arb_in[:]], outs=[arb_out[:]])
                sg = smp.tile([H, 2], f32, tag="sg")
                nc.sync.dma_start(sg[:], arb_out[:])
                # m=sg[:,0]/N; ex2=sg[:,1]/N; var=ex2-m*m; rs=1/sqrt(var+eps)
                mcol = smp.tile([H, 1], f32, tag="mcol")
                nc.vector.tensor_scalar_mul(mcol[:], sg[:, 0:1], 1.0 / N)
                vcol = smp.tile([H, 1], f32, tag="vcol")
                nc.vector.tensor_scalar_mul(vcol[:], sg[:, 1:2], 1.0 / N)
                mmc = smp.tile([H, 1], f32, tag="mmc")
                nc.vector.tensor_mul(mmc[:], mcol[:], mcol[:])
                nc.vector.tensor_sub(vcol[:], vcol[:], mmc[:])
                nc.scalar.activation(vcol[:], vcol[:], Act.Sqrt, bias=epsc[:])
                nc.vector.reciprocal(vcol[:], vcol[:])          # rs
                acol = smp.tile([H, 1], f32, tag="acol")
                nc.vector.tensor_mul(acol[:], vcol[:], gb_s[:, l:l + 1])             # alpha
                ccol = smp.tile([H, 1], f32, tag="ccol")
                nc.vector.tensor_mul(ccol[:], mcol[:], acol[:])
                nc.vector.tensor_sub(ccol[:], gb_s[:, 4 + l:5 + l], ccol[:])         # c2

                if l < 3:
                    # BN+ReLU in f-major: a_T = Relu(ztT*alpha + c2), single ACT op
                    nc.scalar.activation(aT_pre[:], aT_pre[:], Act.Relu,
                                         bias=ccol[:], scale=acol[:])
                    nc.vector.memset(aT_pre[:, A - 32:A], 0.0)   # forced-zero pad slots
                    aT_cur = aT_pre
                else:
                    # node-major apply for pooling: broadcast alpha/c2 to [128, H]
                    arp = psS.tile([1, H], f32, tag="bcps")
                    nc.tensor.transpose(arp[:], acol[:], identf[0:H, 0:H])
                    arow = smp.tile([1, H], f32, tag="arow")
                    nc.vector.tensor_copy(arow[:], arp[:])
                    crp = psS.tile([1, H], f32, tag="bcps")
                    nc.tensor.transpose(crp[:], ccol[:], identf[0:H, 0:H])
                    crow = smp.tile([1, H], f32, tag="crow")
                    nc.vector.tensor_copy(crow[:], crp[:])
                    bcp = psS.tile([128, 2 * H], f32, tag="bcps")
                    nc.tensor.matmul(bcp[:, 0:H], ones_s[:], arow[:], start=True, stop=True)
                    nc.tensor.matmul(bcp[:, H:2 * H], ones_s[:], crow[:], start=True, stop=True)
                    abc = smp.tile([128, 2 * H], f32, tag="abc")
                    nc.vector.tensor_copy(abc[:], bcp[:])
                    nc.vector.tensor_tensor(out=acc0[:], in0=zt[:], in1=rep_free(abc[:, 0:H], B), op=Alu.mult)
                    nc.vector.tensor_tensor(out=acc0[:], in0=acc0[:], in1=rep_free(abc[:, H:2 * H], B), op=Alu.add)
                    a_cur = wk.tile([128, B, H], bf16, tag="a")
                    nc.scalar.activation(a_cur[:], acc0[:], Act.Relu, bias=zcol[:])
                    nc.vector.memset(a_cur[96:128, B - 1:B], 0.0)   # forced-zero pad slots incl 12799

            # ---- pooling: per-graph mean over this core's 64 graphs ----
            pc = psS.tile([GW, H + 1], f32, tag="poolcnt")
            poolp = pc[:, 0:H]
            cntp = pc[:, H:H + 1]
            for b in range(B):
                oh = smp.tile([128, H], bf16, tag="oh", bufs=4)
                nc.vector.tensor_scalar(oh[:], iota_s[:], gl_s[:, b:b + 1], None, Alu.is_equal)
                nc.tensor.matmul(poolp, oh[:], a_cur[:, b], start=(b == 0), stop=(b == B - 1))
                nc.tensor.matmul(cntp, oh[:], onescb_s[:], start=(b == 0), stop=(b == B - 1))
            sums = smp.tile([GW, H], f32, tag="sums")
            nc.vector.tensor_copy(sums[:], poolp)
            cnts = smp.tile([GW, 1], f32, tag="cnts")
            nc.vector.tensor_copy(cnts[:], cntp)
            nc.vector.tensor_scalar_max(cnts[:], cnts[:], 1.0)
            nc.vector.reciprocal(cnts[:], cnts[:])
            nc.vector.tensor_scalar(sums[:], sums[:], cnts[:], None, Alu.mult)
            # pooledT
            ptp = psS.tile([H, GW], f32, tag="stps")
            nc.tensor.transpose(ptp[:], sums[:], identf[0:GW, 0:GW])
            pooledT = smp.tile([H, GW], f32, tag="pooledT")
            nc.vector.tensor_copy(pooledT[:], ptp[:])
            # logits = fcb + pooled @ fcW
            lgp = psS.tile([GW, C], f32, tag="bcps")
            nc.tensor.matmul(lgp[:], ones_s[:, 0:GW], fcb_s[:], start=True, stop=False)
            nc.tensor.matmul(lgp[:], pooledT[:], fcw_s[:], start=False, stop=True)
            logits = smp.tile([GW, C], f32, tag="logits")
            nc.vector.tensor_copy(logits[:], lgp[:])
            mx = smp.tile([GW, 1], f32, tag="mx")
            nc.vector.tensor_reduce(mx[:], logits[:], axis=mybir.AxisListType.X, op=Alu.max)
            nmx = smp.tile([GW, 1], f32, tag="nmx")
            nc.vector.tensor_scalar_mul(nmx[:], mx[:], -1.0)
            et = smp.tile([GW, C], f32, tag="et")
            sume = smp.tile([GW, 1], f32, tag="sume")
            nc.scalar.activation(et[:], logits[:], Act.Exp, bias=nmx[:], accum_out=sume[:])
            lse = smp.tile([GW, 1], f32, tag="lse")
            nc.scalar.activation(lse[:], sume[:], Act.Ln, bias=zcol[0:GW, :])
            res = smp.tile([GW, C], f32, tag="res")
            nc.vector.tensor_scalar(res[:], logits[:], mx[:], lse[:], Alu.subtract, Alu.subtract)
            nc.sync.dma_start(t_out[:], res[:])

    nc.compile()
    return nc


def kernel(x, edge_index, batch, W1, b1, g1, bt1, W2, b2, g2, bt2,
           W3, b3, g3, bt3, W4, b4, g4, bt4, fcW, fcb, **_unused):
    global LAST_EXEC_NS
    hp = _host_prepare(x, edge_index, batch)
    nc = _build_program(hp["addruns"])

    gb = np.zeros((H, 8), np.float32)
    for i, (g, bt) in enumerate(((g1, bt1), (g2, bt2), (g3, bt3), (g4, bt4))):
        gb[:, i] = np.asarray(g, np.float32)
        gb[:, 4 + i] = np.asarray(bt, np.float32)
    w234 = np.concatenate([np.asarray(w, np.float32) for w in (W2, W3, W4)], axis=1)
    iota = np.tile(np.arange(H, dtype=np.float32)[None, :], (128, 1))
    common = {
        "w1": np.asarray(W1, np.float32).astype(np.dtype("bfloat16") if False else np.float32),
        "gb": gb, "fcw": np.asarray(fcW, np.float32), "fcb": np.asarray(fcb, np.float32).reshape(1, C),
        "iota": iota,
        "ones": np.ones((1, 128), np.float32),
        "onesc": np.ones((128, 1), np.float32),
        "ident": np.eye(128, dtype=np.float32),
    }
    import ml_dtypes
    bfl = ml_dtypes.bfloat16
    in_maps = []
    for k in range(NCORES):
        m = {
            "xT": hp["xT"][k].astype(bfl),
            "w1": np.asarray(W1, np.float32).astype(bfl),
            "w234": w234.astype(bfl),
            "gb": gb, "fcw": common["fcw"], "fcb": common["fcb"],
            "deg": hp["deg_loc"][k], "gl": hp["g_loc"][k],
            "iota": iota, "ones": common["ones"], "onesc": common["onesc"],
            "onescb": np.ones((128, 1), bfl),
            "ident": np.eye(128, dtype=np.float32).astype(bfl),
            "idx": hp["idx_wrapped"][k], "midx": hp["merge_idx"][k],
        }
        in_maps.append(m)

    res = run_bass_kernel_spmd(nc, in_maps, core_ids=list(range(NCORES)),
                               trace=os.environ.get("GCN_TRACE", "0") == "1")
    LAST_EXEC_NS = res.exec_time_ns
    out = np.concatenate([res.results[k]["out"] for k in range(NCORES)], axis=0)
    return out.astype(np.float32)


# revision 13
# speedup vs baseline: 2.4950x; 1.0249x over previous
"""GCN (4-layer GCNConv + BN/ReLU + mean-pool + FC + log_softmax) on 8 Trainium2 NeuronCores.

Sharding: nodes partitioned into 8 cores by contiguous 64-graph windows (graph parallel);
edges partitioned by destination core. Per layer: local matmul -> AllGather of the
dis-scaled feature table -> dma_gather edge aggregation into 4 per-source-chunk
accumulators (ELL-style pass schedule, per-chunk degree-sorted slot orderings) ->
merge -> BN (stats AllReduce) + ReLU. Final: one-hot matmul pooling + FC + log_softmax.
"""
import sys, types, os
import numpy as np


def _install_axon_hooks():
    if "antenv.axon_hooks" in sys.modules:
        return
    try:
        import antenv
    except ImportError:
        return
    mod = types.ModuleType("antenv.axon_hooks")
    state = {"hook": None}
    mod.set_axon_ntff_profile_hook = lambda h: state.__setitem__("hook", h)
    mod.get_axon_ntff_profile_hook = lambda: state["hook"]
    sys.modules["antenv.axon_hooks"] = mod
    antenv.axon_hooks = mod
    try:
        from trn_agent_boot.trn_boot import _ntff_profile_via_ctypes
        state["hook"] = _ntff_profile_via_ctypes("/opt/axon/libaxon_pjrt.so")
    except Exception:
        pass


_install_axon_hooks()

import concourse.bacc as bacc
import concourse.bass as bass
import concourse.mybir as mybir
import concourse.tile as tile
from concourse.ap import AP
from concourse.library_config import mlp
from concourse.bass_utils import run_bass_kernel_spmd

# ---- static problem shapes ----
N = 100000
E = 1600000
G = 512
FIN = 128
H = 64
C = 10
EPS = 1e-5
NCORES = 8
A = 12800            # slots per core (100 blocks of 128)
B = A // 128         # 100 blocks
GW = G // NCORES     # 64 graphs per core
NCHUNK = 4           # source chunks (pairs of cores), 25600 rows each
CHROWS = 2 * A       # rows per source chunk
CALL = 1024          # idxs per dma_gather call (single_packet limit)
NCALLS = 52          # calls per chunk per layer (52*1024 = 53248 >= max padded rows)
MCALLS = 13          # merge gather calls (13*1024 >= 12800)
DUMMY = CHROWS - 1   # in-chunk dummy row index (forced-zero slot 12799 of 2nd core)
PAD_DEG = 1.0e38

LAST_EXEC_NS = None

f32 = mybir.dt.float32
bf16 = mybir.dt.bfloat16
i16 = mybir.dt.int16
Alu = mybir.AluOpType
Act = mybir.ActivationFunctionType


def _wrap_idx(v):
    """int16 idx vector (len mult of 16) -> [128, len/16] wrapped+replicated layout."""
    blk = v.reshape(-1, 16).T.astype(np.int16)
    return np.tile(blk, (8, 1))


def _host_prepare(x, edge_index, batch):
    src = np.asarray(edge_index[0], np.int64)
    dst = np.asarray(edge_index[1], np.int64)
    batch = np.asarray(batch, np.int64)
    gsize = np.bincount(batch, minlength=G)
    gw_nodes = gsize.reshape(NCORES, GW).sum(1)
    assert gw_nodes.max() <= A, f"core node count {gw_nodes.max()} exceeds {A} slots"
    node_off = np.concatenate([[0], np.cumsum(gw_nodes)])
    core_of_node = np.repeat(np.arange(NCORES), gw_nodes)

    dst_core = core_of_node[dst]
    src_chunk = core_of_node[src] // 2

    # per (core, chunk) multiplicity of each node
    cnt = np.zeros((NCHUNK, N), np.int64)
    for q in range(NCHUNK):
        m = src_chunk == q
        cnt[q] += np.bincount(dst[m], minlength=N)
    cnt_tot = cnt.sum(0)

    # orderings: primary = chunk0-sorted; slotq = chunk-q sorted (per core)
    slot_p = np.full(N, -1, np.int64)      # node -> primary slot (0..A)
    node_of_slot = np.full((NCORES, A), -1, np.int64)
    slot_q = np.full((NCHUNK, N), -1, np.int64)
    order_q_all = {}
    for k in range(NCORES):
        nodes_k = np.arange(node_off[k], node_off[k + 1])
        for q in range(NCHUNK):
            order = nodes_k[np.argsort(-cnt[q][nodes_k], kind="stable")]
            slot_q[q][order] = np.arange(len(order))
            order_q_all[(k, q)] = order
        slot_p[order_q_all[(k, 0)]] = np.arange(len(nodes_k))
        node_of_slot[k, :len(nodes_k)] = order_q_all[(k, 0)]

    # common pass lengths L[q][j] (max over cores, 128-aligned)
    npass = np.zeros(NCHUNK, np.int64)
    nact = np.zeros((NCORES, NCHUNK, 64), np.int64)
    for k in range(NCORES):
        for q in range(NCHUNK):
            nodes_k = np.arange(node_off[k], node_off[k + 1])
            cq = cnt[q][nodes_k]
            mx = int(cq.max()) if len(cq) else 0
            npass[q] = max(npass[q], mx)
            for j in range(mx):
                nact[k, q, j] = int((cq > j).sum())
    Lpad = [[int(np.ceil(nact[:, q, j].max() / 128) * 128) for j in range(npass[q])]
            for q in range(NCHUNK)]
    for q in range(NCHUNK):
        assert sum(Lpad[q]) <= NCALLS * CALL, (q, sum(Lpad[q]))

    # gather index arrays [NCHUNK, NCALLS, 128, CALL//16] + add schedule
    idx_all = np.zeros((NCORES, NCHUNK, NCALLS * CALL), np.int16)
    idx_all[:] = DUMMY
    # table row of a source node, within its chunk: (core%2)*A + primary slot
    row_in_chunk = (core_of_node % 2) * A + (slot_p % 128) * B + slot_p // 128
    sched = [[] for _ in range(NCHUNK)]  # per chunk: list of (pos_blk, acc_blk, nblk) common
    for q in range(NCHUNK):
        pos = 0
        for j, L in enumerate(Lpad[q]):
            sched[q].append((pos // 128, 0, L // 128, j))
            pos += L
    for k in range(NCORES):
        ek = dst_core == k
        s_k, d_k = src[ek], dst[ek]
        cq_k = src_chunk[ek]
        for q in range(NCHUNK):
            m = cq_k == q
            s_q, d_q = s_k[m], d_k[m]
            dsl = slot_q[q][d_q]
            o = np.argsort(dsl, kind="stable")
            s_q, dsl = s_q[o], dsl[o]
            # j-th edge of each slot: rank within equal dsl run
            jrank = np.arange(len(dsl)) - np.searchsorted(dsl, dsl)
            pos0 = np.concatenate([[0], np.cumsum([L for L in Lpad[q]])])
            flat = pos0[jrank] + dsl
            idx_all[k, q, flat] = row_in_chunk[s_q].astype(np.int16)

    idx_wrapped = np.zeros((NCORES, NCHUNK, NCALLS, 128, CALL // 16), np.int16)
    for k in range(NCORES):
        for q in range(NCHUNK):
            for c in range(NCALLS):
                idx_wrapped[k, q, c] = _wrap_idx(idx_all[k, q, c * CALL:(c + 1) * CALL])

    # per-call add schedule (common): call c covers staging blocks -> acc block ranges
    addsched = [[[] for _ in range(NCALLS)] for _ in range(NCHUNK)]
    for q in range(NCHUNK):
        pos = 0
        for j, L in enumerate(Lpad[q]):
            for blk in range(L // 128):
                g_abs = pos // 128 + blk
                addsched[q][g_abs // 8].append((g_abs % 8, blk))
            pos += L
    # compress consecutive runs: list of (st_blk0, acc_blk0, n)
    addruns = [[[] for _ in range(NCALLS)] for _ in range(NCHUNK)]
    for q in range(NCHUNK):
        for c in range(NCALLS):
            for st_b, ac_b in addsched[q][c]:
                runs = addruns[q][c]
                if runs and runs[-1][0] + runs[-1][2] == st_b and runs[-1][1] + runs[-1][2] == ac_b:
                    runs[-1][2] += 1
                else:
                    runs.append([st_b, ac_b, 1])

    # merge permutation: z[primary slot s] += acc_q[slot_q of node at s]
    merge_idx = np.zeros((NCORES, NCHUNK - 1, MCALLS, 128, CALL // 16), np.int16)
    for k in range(NCORES):
        nk = int(gw_nodes[k])
        for q in range(1, NCHUNK):
            mi = np.arange(A, dtype=np.int64)
            mi[:nk] = slot_q[q][node_of_slot[k, :nk]]
            mi = (mi % 128) * B + mi // 128
            mi = np.concatenate([mi, np.zeros(MCALLS * CALL - A, np.int64)])
            for c in range(MCALLS):
                merge_idx[k, q - 1, c] = _wrap_idx(mi[c * CALL:(c + 1) * CALL].astype(np.int16))

    # per-core per-slot data
    xT = np.zeros((NCORES, FIN, A), np.float32)
    deg_loc = np.full((NCORES, 128, B), PAD_DEG, np.float32)
    g_loc = np.full((NCORES, 128, B), -1.0, np.float32)
    for k in range(NCORES):
        nk = int(gw_nodes[k])
        nodes = node_of_slot[k, :nk]
        sl = np.arange(nk)
        xT[k][:, sl] = np.asarray(x, np.float32)[nodes].T
        p, bb = sl % 128, sl // 128
        deg_loc[k][p, bb] = cnt_tot[nodes].astype(np.float32)
        g_loc[k][p, bb] = (batch[nodes] - k * GW).astype(np.float32)

    return dict(gw_nodes=gw_nodes, node_off=node_off, idx_wrapped=idx_wrapped,
                addruns=addruns, merge_idx=merge_idx, xT=xT, deg_loc=deg_loc,
                g_loc=g_loc, Lpad=Lpad)


def _build_program(addruns):
    nc = bacc.Bacc("TRN2", target_bir_lowering=False, debug=False,
                   num_devices=NCORES, num_swdge_queues=4)

    # inputs
    t_xT = nc.dram_tensor("xT", [FIN, A], bf16, kind="ExternalInput")
    t_w1 = nc.dram_tensor("w1", [FIN, H], bf16, kind="ExternalInput")
    t_w = nc.dram_tensor("w234", [H, 3 * H], bf16, kind="ExternalInput")
    t_gb = nc.dram_tensor("gb", [H, 8], f32, kind="ExternalInput")
    t_fcw = nc.dram_tensor("fcw", [H, C], f32, kind="ExternalInput")
    t_fcb = nc.dram_tensor("fcb", [1, C], f32, kind="ExternalInput")
    t_deg = nc.dram_tensor("deg", [128, B], f32, kind="ExternalInput")
    t_gl = nc.dram_tensor("gl", [128, B], f32, kind="ExternalInput")
    t_iota = nc.dram_tensor("iota", [128, H], f32, kind="ExternalInput")
    t_ones = nc.dram_tensor("ones", [1, 128], f32, kind="ExternalInput")
    t_onesc = nc.dram_tensor("onesc", [128, 1], f32, kind="ExternalInput")
    t_onescb = nc.dram_tensor("onescb", [128, 1], bf16, kind="ExternalInput")
    t_ident = nc.dram_tensor("ident", [128, 128], bf16, kind="ExternalInput")
    t_idx = nc.dram_tensor("idx", [NCHUNK, NCALLS, 128, CALL // 16], i16, kind="ExternalInput")
    t_midx = nc.dram_tensor("midx", [NCHUNK - 1, MCALLS, 128, CALL // 16], i16, kind="ExternalInput")
    t_out = nc.dram_tensor("out", [GW, C], f32, kind="ExternalOutput")

    with tile.TileContext(nc) as tc:
        with tc.tile_pool(name="const", bufs=1) as cst, \
             tc.tile_pool(name="accp", bufs=1) as accp, \
             tc.tile_pool(name="work", bufs=1) as wk, \
             tc.tile_pool(name="stage", bufs=12) as stp, \
             tc.tile_pool(name="idxp", bufs=12) as idp, \
             tc.tile_pool(name="small", bufs=2) as smp, \
             tc.tile_pool(name="psA", bufs=2, space="PSUM") as psA, \
             tc.tile_pool(name="psB", bufs=2, space="PSUM") as psB, \
             tc.tile_pool(name="psS", bufs=1, space="PSUM") as psS, \
             tc.tile_pool(name="dram", bufs=1, space="DRAM") as drp:

            nc.gpsimd.load_library(mlp)

            # constants to SBUF
            xT_s = wk.tile([FIN, A], bf16, tag="big2")
            nc.sync.dma_start(xT_s[:], t_xT[:])
            w1_s = cst.tile([FIN, H], bf16)
            nc.sync.dma_start(w1_s[:], t_w1[:])
            w_s = cst.tile([H, 3 * H], bf16)
            nc.sync.dma_start(w_s[:], t_w[:])
            gb_s = cst.tile([H, 8], f32)
            nc.sync.dma_start(gb_s[:], t_gb[:])
            fcw_s = cst.tile([H, C], f32)
            nc.sync.dma_start(fcw_s[:], t_fcw[:])
            fcb_s = cst.tile([1, C], f32)
            nc.sync.dma_start(fcb_s[:], t_fcb[:])
            deg_s = cst.tile([128, B], f32)
            nc.sync.dma_start(deg_s[:], t_deg[:])
            gl_s = cst.tile([128, B], f32)
            nc.sync.dma_start(gl_s[:], t_gl[:])
            iota_s = cst.tile([128, H], f32)
            nc.sync.dma_start(iota_s[:], t_iota[:])
            ones_s = cst.tile([1, 128], f32)
            nc.sync.dma_start(ones_s[:], t_ones[:])
            onesc_s = cst.tile([128, 1], f32)
            nc.sync.dma_start(onesc_s[:], t_onesc[:])
            onescb_s = cst.tile([128, 1], bf16)
            nc.sync.dma_start(onescb_s[:], t_onescb[:])
            ident_s = cst.tile([128, 128], bf16)
            nc.sync.dma_start(ident_s[:], t_ident[:])

            zcol = cst.tile([128, 1], f32)
            nc.vector.memset(zcol[:], 0.0)
            epsc = cst.tile([H, 1], f32)
            nc.vector.memset(epsc[:], EPS)
            identf = cst.tile([128, 128], f32)
            nc.vector.tensor_copy(identf[:], ident_s[:])
            # dis = 1/sqrt(deg+1)
            dis_s = cst.tile([128, B], f32)
            nc.scalar.activation(dis_s[:], deg_s[:], Act.Sqrt, bias=onesc_s[:])
            nc.vector.reciprocal(dis_s[:], dis_s[:])

            def bcast_dis(bsl):  # dis slice [128, nb] -> AP [128, nb, 64]
                s = dis_s[:, bsl]
                return AP(s.tensor, s.offset, [s.ap[0], s.ap[1], [0, H]])

            def rep_free(ap2d, n):  # [P, F] -> [P, n, F] (free repeat)
                return AP(ap2d.tensor, ap2d.offset, [ap2d.ap[0], [0, n], ap2d.ap[1]])

            a_cur = None       # [128, B, H] bf16 post-BN activation (node-major slots)
            aT_cur = None      # [H, A] bf16 transposed

            for l in range(4):
                # ---- local matmul: h_scaled_local = (a @ W) * dis ----
                acc0 = accp.tile([128, B, H], f32, tag="acc0")
                K = FIN if l == 0 else H
                lhsT_full = xT_s if l == 0 else aT_cur
                W_ap = w1_s[:] if l == 0 else w_s[:, (l - 1) * H:l * H]
                for b8 in range(0, B, 8):
                    nblk = min(8, B - b8)
                    pt = psA.tile([128, 8, H], f32, tag="mmps")
                    for bb in range(nblk):
                        nc.tensor.matmul(pt[:, bb], lhsT_full[:, (b8 + bb) * 128:(b8 + bb + 1) * 128],
                                         W_ap, start=True, stop=True)
                    nc.vector.tensor_tensor(out=acc0[:, b8:b8 + nblk], in0=pt[:, 0:nblk],
                                            in1=bcast_dis(slice(b8, b8 + nblk)), op=Alu.mult)
                # shard -> DRAM (row-major by slot: row s=(p + 128*b))
                shard = drp.tile([A, H], f32, tag="shard")
                sh_ap = AP(shard[:].tensor, shard[:].offset,
                           [[B * H, 128], [1, B * H]])
                nc.sync.dma_start(sh_ap, acc0[:].rearrange("p b f -> p (b f)"))
                table = drp.tile([NCORES * A, H], f32, tag="table", addr_space="Shared")
                nc.gpsimd.collective_compute(
                    "AllGather", Alu.bypass,
                    replica_groups=[list(range(NCORES))],
                    ins=[shard[:]], outs=[table[:]])

                # ---- edge gathers into 4 accumulators ----
                accq = [acc0]
                for q in range(1, NCHUNK):
                    aq = accp.tile([128, B, H], f32, tag=f"acc{q}")
                    nc.vector.memset(aq[:], 0.0)
                    accq.append(aq)
                for c in range(NCALLS):
                    for q in range(NCHUNK):
                        if not addruns[q][c]:
                            continue
                        src_ap = table[q * CHROWS:(q + 1) * CHROWS, :]
                        it = idp.tile([128, CALL // 16], i16, tag="idx")
                        nc.sync.dma_start(it[:], t_idx[q, c])
                        st = stp.tile([128, 8, H], f32, tag="stage")
                        nc.gpsimd.dma_gather(st[:], src_ap, it[:], CALL, CALL, H,
                                             single_packet=True, queue_num=q)
                        for st_b, ac_b, nb in addruns[q][c]:
                            nc.vector.tensor_add(accq[q][:, ac_b:ac_b + nb],
                                                 accq[q][:, ac_b:ac_b + nb],
                                                 st[:, st_b:st_b + nb])
                # ---- merge acc1..3 into acc0 (permuted via local gather) ----
                scrs = {}
                for q in range(1, NCHUNK):
                    scr = drp.tile([A, H], f32, tag=f"scr{q}")
                    sc_ap = AP(scr[:].tensor, scr[:].offset,
                               [[B * H, 128], [1, B * H]])
                    nc.sync.dma_start(sc_ap, accq[q][:].rearrange("p b f -> p (b f)"))
                    scrs[q] = scr
                for c in range(MCALLS):
                    for q in range(1, NCHUNK):
                        nblk = min(8, B - c * 8)
                        it = idp.tile([128, CALL // 16], i16, tag="idx")
                        nc.sync.dma_start(it[:], t_midx[q - 1, c])
                        st = stp.tile([128, 8, H], f32, tag="stage")
                        nc.gpsimd.dma_gather(st[:], scrs[q][:], it[:], CALL, CALL, H,
                                             single_packet=True, queue_num=(q + c) % 4)
                        nc.vector.tensor_add(acc0[:, c * 8:c * 8 + nblk],
                                             acc0[:, c * 8:c * 8 + nblk], st[:, 0:nblk])

                # ---- zt = acc0 * dis ; stats; BN+ReLU ----
                nc.vector.tensor_tensor(out=acc0[:], in0=acc0[:], in1=bcast_dis(slice(0, B)), op=Alu.mult)
                zt = acc0
                s1t = smp.tile([128, H], f32, tag="s1")
                nc.vector.tensor_reduce(s1t[:], zt[:].rearrange("p b f -> p f b"),
                                        axis=mybir.AxisListType.X, op=Alu.add)
                sq = wk.tile([128, B, H], f32, tag="big2")
                nc.vector.tensor_mul(sq[:], zt[:], zt[:])
                s2t = smp.tile([128, H], f32, tag="s2")
                nc.vector.tensor_reduce(s2t[:], sq[:].rearrange("p b f -> p f b"),
                                        axis=mybir.AxisListType.X, op=Alu.add)
                spt = psS.tile([H, 2], f32, tag="stps")
                nc.tensor.matmul(spt[:, 0:1], s1t[:], onesc_s[:], start=True, stop=True)
                nc.tensor.matmul(spt[:, 1:2], s2t[:], onesc_s[:], start=True, stop=True)
                # transpose zt -> aT_pre (bf16 f-major) BEFORE the AllReduce (overlaps AR)
                if l < 3:
                    aT_pre = wk.tile([H, A], bf16, tag="big2")
                    for b4 in range(0, B, 4):
                        nblk = min(4, B - b4)
                        tpf = psB.tile([H, 4, 128], f32, tag="trps")
                        for bb in range(nblk):
                            nc.tensor.transpose(tpf[:, bb], zt[:, b4 + bb], identf[:])
                        nc.vector.tensor_copy(aT_pre[:, b4 * 128:(b4 + nblk) * 128],
                                              tpf[:, 0:nblk].rearrange("p b f -> p (b f)"))
                scol = smp.tile([H, 2], f32, tag="scol")
                nc.vector.tensor_copy(scol[:], spt[:])
                arb_in = drp.tile([H, 2], f32, tag="arbin")
                arb_out = drp.tile([H, 2], f32, tag="arbout", addr_space="Shared")
                nc.sync.dma_start(arb_in[:], scol[:])
                nc.gpsimd.collective_compute(
                    "AllReduce", Alu.add,
                    replica_groups=[list(range(NCORES))],
                    ins=[arb_in[:]], outs=[arb_out[:]])
                sg = smp.tile([H, 2], f32, tag="sg")
                nc.sync.dma_start(sg[:], arb_out[:])
                # m=sg[:,0]/N; ex2=sg[:,1]/N; var=ex2-m*m; rs=1/sqrt(var+eps)
                mcol = smp.tile([H, 1], f32, tag="mcol")
                nc.vector.tensor_scalar_mul(mcol[:], sg[:, 0:1], 1.0 / N)
                vcol = smp.tile([H, 1], f32, tag="vcol")
                nc.vector.tensor_scalar_mul(vcol[:], sg[:, 1:2], 1.0 / N)
                mmc = smp.tile([H, 1], f32, tag="mmc")
                nc.vector.tensor_mul(mmc[:], mcol[:], mcol[:])
                nc.vector.tensor_sub(vcol[:], vcol[:], mmc[:])
                nc.scalar.activation(vcol[:], vcol[:], Act.Sqrt, bias=epsc[:])
                nc.vector.reciprocal(vcol[:], vcol[:])          # rs
                acol = smp.tile([H, 1], f32, tag="acol")
                nc.vector.tensor_mul(acol[:], vcol[:], gb_s[:, l:l + 1])             # alpha
                ccol = smp.tile([H, 1], f32, tag="ccol")
                nc.vector.tensor_mul(ccol[:], mcol[:], acol[:])
                nc.vector.tensor_sub(ccol[:], gb_s[:, 4 + l:5 + l], ccol[:])         # c2

                if l < 3:
                    # BN+ReLU in f-major: a_T = Relu(ztT*alpha + c2), single ACT op
                    nc.scalar.activation(aT_pre[:], aT_pre[:], Act.Relu,
                                         bias=ccol[:], scale=acol[:])
                    nc.vector.memset(aT_pre[:, A - 32:A], 0.0)   # forced-zero pad slots
                    aT_cur = aT_pre
                else:
                    # node-major apply for pooling: broadcast alpha/c2 to [128, H]
                    arp = psS.tile([1, H], f32, tag="bcps")
                    nc.tensor.transpose(arp[:], acol[:], identf[0:H, 0:H])
                    arow = smp.tile([1, H], f32, tag="arow")
                    nc.vector.tensor_copy(arow[:], arp[:])
                    crp = psS.tile([1, H], f32, tag="bcps")
                    nc.tensor.transpose(crp[:], ccol[:], identf[0:H, 0:H])
                    crow = smp.tile([1, H], f32, tag="crow")
                    nc.vector.tensor_copy(crow[:], crp[:])
                    bcp = psS.tile([128, 2 * H], f32, tag="bcps")
                    nc.tensor.matmul(bcp[:, 0:H], ones_s[:], arow[:], start=True, stop=True)
                    nc.tensor.matmul(bcp[:, H:2 * H], ones_s[:], crow[:], start=True, stop=True)
                    abc = smp.tile([128, 2 * H], f32, tag="abc")
                    nc.vector.tensor_copy(abc[:], bcp[:])
                    nc.vector.tensor_tensor(out=acc0[:], in0=zt[:], in1=rep_free(abc[:, 0:H], B), op=Alu.mult)
                    nc.vector.tensor_tensor(out=acc0[:], in0=acc0[:], in1=rep_free(abc[:, H:2 * H], B), op=Alu.add)
                    a_cur = wk.tile([128, B, H], bf16, tag="a")
                    nc.scalar.activation(a_cur[:], acc0[:], Act.Relu, bias=zcol[:])
                    nc.vector.memset(a_cur[96:128, B - 1:B], 0.0)   # forced-zero pad slots incl 12799

            # ---- pooling: per-graph mean over this core's 64 graphs ----
            pc = psS.tile([GW, H + 1], f32, tag="poolcnt")
            poolp = pc[:, 0:H]
            cntp = pc[:, H:H + 1]
            for b in range(B):
                oh = smp.tile([128, H], bf16, tag="oh", bufs=4)
                nc.vector.tensor_scalar(oh[:], iota_s[:], gl_s[:, b:b + 1], None, Alu.is_equal)
                nc.tensor.matmul(poolp, oh[:], a_cur[:, b], start=(b == 0), stop=(b == B - 1))
                nc.tensor.matmul(cntp, oh[:], onescb_s[:], start=(b == 0), stop=(b == B - 1))
            sums = smp.tile([GW, H], f32, tag="sums")
            nc.vector.tensor_copy(sums[:], poolp)
            cnts = smp.tile([GW, 1], f32, tag="cnts")
            nc.vector.tensor_copy(cnts[:], cntp)
            nc.vector.tensor_scalar_max(cnts[:], cnts[:], 1.0)
            nc.vector.reciprocal(cnts[:], cnts[:])
            nc.vector.tensor_scalar(sums[:], sums[:], cnts[:], None, Alu.mult)
            # pooledT
            ptp = psS.tile([H, GW], f32, tag="stps")
            nc.tensor.transpose(ptp[:], sums[:], identf[0:GW, 0:GW])
            pooledT = smp.tile([H, GW], f32, tag="pooledT")
            nc.vector.tensor_copy(pooledT[:], ptp[:])
            # logits = fcb + pooled @ fcW
            lgp = psS.tile([GW, C], f32, tag="bcps")
            nc.tensor.matmul(lgp[:], ones_s[:, 0:GW], fcb_s[:], start=True, stop=False)
            nc.tensor.matmul(lgp[:], pooledT[:], fcw_s[:], start=False, stop=True)
            logits = smp.tile([GW, C], f32, tag="logits")
            nc.vector.tensor_copy(logits[:], lgp[:])
            mx = smp.tile([GW, 1], f32, tag="mx")
            nc.vector.tensor_reduce(mx[:], logits[:], axis=mybir.AxisListType.X, op=Alu.max)
            nmx = smp.tile([GW, 1], f32, tag="nmx")
            nc.vector.tensor_scalar_mul(nmx[:], mx[:], -1.0)
            et = smp.tile([GW, C], f32, tag="et")
            sume = smp.tile([GW, 1], f32, tag="sume")
            nc.scalar.activation(et[:], logits[:], Act.Exp, bias=nmx[:], accum_out=sume[:])
            lse = smp.tile([GW, 1], f32, tag="lse")
            nc.scalar.activation(lse[:], sume[:], Act.Ln, bias=zcol[0:GW, :])
            res = smp.tile([GW, C], f32, tag="res")
            nc.vector.tensor_scalar(res[:], logits[:], mx[:], lse[:], Alu.subtract, Alu.subtract)
            nc.sync.dma_start(t_out[:], res[:])

    nc.compile()
    return nc


def kernel(x, edge_index, batch, W1, b1, g1, bt1, W2, b2, g2, bt2,
           W3, b3, g3, bt3, W4, b4, g4, bt4, fcW, fcb, **_unused):
    global LAST_EXEC_NS
    hp = _host_prepare(x, edge_index, batch)
    nc = _build_program(hp["addruns"])

    gb = np.zeros((H, 8), np.float32)
    for i, (g, bt) in enumerate(((g1, bt1), (g2, bt2), (g3, bt3), (g4, bt4))):
        gb[:, i] = np.asarray(g, np.float32)
        gb[:, 4 + i] = np.asarray(bt, np.float32)
    w234 = np.concatenate([np.asarray(w, np.float32) for w in (W2, W3, W4)], axis=1)
    iota = np.tile(np.arange(H, dtype=np.float32)[None, :], (128, 1))
    common = {
        "w1": np.asarray(W1, np.float32).astype(np.dtype("bfloat16") if False else np.float32),
        "gb": gb, "fcw": np.asarray(fcW, np.float32), "fcb": np.asarray(fcb, np.float32).reshape(1, C),
        "iota": iota,
        "ones": np.ones((1, 128), np.float32),
        "onesc": np.ones((128, 1), np.float32),
        "ident": np.eye(128, dtype=np.float32),
    }
    import ml_dtypes
    bfl = ml_dtypes.bfloat16
    in_maps = []
    for k in range(NCORES):
        m = {
            "xT": hp["xT"][k].astype(bfl),
            "w1": np.asarray(W1, np.float32).astype(bfl),
            "w234": w234.astype(bfl),
            "gb": gb, "fcw": common["fcw"], "fcb": common["fcb"],
            "deg": hp["deg_loc"][k], "gl": hp["g_loc"][k],
            "iota": iota, "ones": common["ones"], "onesc": common["onesc"],
            "onescb": np.ones((128, 1), bfl),
            "ident": np.eye(128, dtype=np.float32).astype(bfl),
            "idx": hp["idx_wrapped"][k], "midx": hp["merge_idx"][k],
        }
        in_maps.append(m)

    res = run_bass_kernel_spmd(nc, in_maps, core_ids=list(range(NCORES)),
                               trace=os.environ.get("GCN_TRACE", "0") == "1")
    LAST_EXEC_NS = res.exec_time_ns
    out = np.concatenate([res.results[k]["out"] for k in range(NCORES)], axis=0)
    return out.astype(np.float32)
